# revision 1
# baseline (speedup 1.0000x reference)
"""Trainium2 Bass kernel for nn_DeformableAttention (B=4, C=384, H=W=56, NH=12, HC=32, STRIDE=2).

Self-contained: hardcodes shapes/sharding. Sharding: 8 cores = 4 batches x 2
pixel-row-halves. Each core computes the full value/key/offset branches for its
batch (duplicated across the pair) and the query branch + final GEMM for its
half of the 3136 output pixels.

Math note: the reference computes out = (scale * q^T k) v^T without softmax, so
attention is linear and reassociates:
    y[b] = (w_out @ blockdiag_h(scale * M[b,h])) @ Q[b],
    M[b,h] = V_s[b,h] K[b,h]^T  (32x32 per head)
which drops the 48x(3136x784x32) einsums to a few small GEMMs.
"""
import contextlib

import numpy as np

import concourse.bass as bass
import concourse.tile as tile
from concourse import bacc, mybir
from concourse.bass_utils import run_bass_kernel_spmd
from concourse.masks import make_identity

F32, F16, I32 = mybir.dt.float32, mybir.dt.float16, mybir.dt.int32
MULT, ADD, SUB = mybir.AluOpType.mult, mybir.AluOpType.add, mybir.AluOpType.subtract
AF = mybir.ActivationFunctionType

B, C, H, W = 4, 384, 56, 56
NH, HC = 12, 32
SCALE = HC ** -0.5
HP = H + 2                      # 58 padded
PIX = H * W                     # 3136
PIXPAD = 3200                   # padded to xbar 128-multiple
KH = KW = 28                    # stride-2 output
N = KH * KW                     # 784
NT = 112                        # point-tile size (7 tiles)
NTILES = N // NT
HALF_ROWS = H // 2              # 28
HALF_PIX = HALF_ROWS * W        # 1568
CT = C // 128                   # 3 channel tiles
EPS = 1e-5

_CACHE = {}


def _emit(nc, tc, ctx, io):
    pool = ctx.enter_context(tc.tile_pool(name="main", bufs=1))
    st32 = tc.tile_pool(name="stage32", bufs=1)
    st32p = st32.__enter__()
    dma = nc.sync

    # ---------------- loads ----------------
    xp32 = []
    for ct in range(CT):
        t = st32p.tile([128, HP * HP], F32, tag=f"xp32_{ct}")
        dma.dma_start(t[:], io["xp"][ct * 128:(ct + 1) * 128, :])
        xp32.append(t)
    xp16 = []
    for ct in range(CT):
        t = pool.tile([128, HP * HP], F16, tag=f"xp16_{ct}")
        nc.vector.tensor_copy(t[:], xp32[ct][:])
        xp16.append(t)
    xq16 = []
    for ct in range(CT):
        f = st32p.tile([128, 30 * HP], F32, tag=f"xq32_{ct}")
        dma.dma_start(f[:], io["xq"][ct * 128:(ct + 1) * 128, :])
        t = pool.tile([128, 30 * HP], F16, tag=f"xq16_{ct}")
        nc.vector.tensor_copy(t[:], f[:])
        xq16.append(t)

    def load_cols(name, width, dtype=F32):
        out = []
        for ct in range(CT):
            t = pool.tile([128, width], dtype, tag=f"{name}_{ct}")
            dma.dma_start(t[:], io[name][ct * 128:(ct + 1) * 128, :])
            out.append(t)
        return out

    wv = load_cols("wv", 9)
    wq = load_cols("wq", 9)
    wk = load_cols("wk", 9)
    wo = load_cols("wo", 9)
    bv = load_cols("bv", 1)
    bq = load_cols("bq", 1)
    bk = load_cols("bk", 1)
    bo = load_cols("bo", 1)
    lng = load_cols("lng", 1)
    lnb = load_cols("lnb", 1)
    w2t = load_cols("w2t", 2)
    wot32 = load_cols("wot", C)
    wot16 = []
    for ct in range(CT):
        t = pool.tile([128, C], F16, tag=f"wot16_{ct}")
        nc.vector.tensor_copy(t[:], wot32[ct][:])
        wot16.append(t)
    refyx = pool.tile([2, N], F32, tag="refyx")
    dma.dma_start(refyx[:], io["refyx"][:, :])
    ones = pool.tile([128, 1], F32, tag="ones")
    nc.vector.memset(ones[:], 1.0)
    ident = pool.tile([128, 128], F16, tag="ident")
    make_identity(nc, ident[:])

    # ---------------- conv helper ----------------
    def dwconv(eng, out2, xt, w, b, base_row, stride, rows, cols):
        # out2: [128, rows, cols] view; xt: [128, hp, 58] view (fp tile)
        for t in range(9):
            dy, dx = t // 3, t % 3
            r0 = base_row + dy
            src = xt[:, r0:r0 + (rows - 1) * stride + 1:stride,
                     dx:dx + (cols - 1) * stride + 1:stride]
            if t == 0:
                eng.tensor_scalar(out=out2, in0=src, scalar1=w[:, 0:1],
                                  scalar2=b[:, 0:1], op0=MULT, op1=ADD)
            else:
                eng.scalar_tensor_tensor(out=out2, in0=src, scalar=w[:, t:t + 1],
                                         in1=out2, op0=MULT, op1=ADD)

    # ---------------- off branch (fp32, critical path) ----------------
    off = []
    for ct in range(CT):
        t = pool.tile([128, N], F32, tag=f"off_{ct}")
        x3 = xp32[ct][:].rearrange("p (h w) -> p h w", h=HP)
        dwconv(nc.vector, t[:].rearrange("p (h w) -> p h w", h=KH),
               x3, wo[ct], bo[ct], 0, 2, KH, KW)
        off.append(t)
    st32.__exit__(None, None, None)

    with tc.tile_pool(name="ln_psum", bufs=1, space="PSUM") as lnp:
        mu_ps = lnp.tile([1, N], F32, tag="mu")
        ssq_ps = lnp.tile([1, N], F32, tag="ssq")
        sq = []
        for ct in range(CT):
            t = pool.tile([128, N], F32, tag=f"sq_{ct}")
            nc.scalar.activation(t[:], off[ct][:], AF.Square)
            sq.append(t)
        for sl in (slice(0, 512), slice(512, N)):
            for ct in range(CT):
                nc.tensor.matmul(mu_ps[:, sl], ones[:], off[ct][:, sl],
                                 start=(ct == 0), stop=(ct == CT - 1))
            for ct in range(CT):
                nc.tensor.matmul(ssq_ps[:, sl], ones[:], sq[ct][:, sl],
                                 start=(ct == 0), stop=(ct == CT - 1))
        # stats [1, N]
        mu = pool.tile([1, N], F32, tag="mu_sb")
        nc.scalar.activation(mu[:], mu_ps[:], AF.Copy, scale=1.0 / C)
        es = pool.tile([1, N], F32, tag="es_sb")
        nc.scalar.activation(es[:], ssq_ps[:], AF.Copy, scale=1.0 / C)
    musq = pool.tile([1, N], F32, tag="musq")
    nc.scalar.activation(musq[:], mu[:], AF.Square)
    var = pool.tile([1, N], F32, tag="var")
    nc.vector.tensor_tensor(out=var[:], in0=es[:], in1=musq[:], op=SUB)
    nc.vector.tensor_scalar_add(var[:], var[:], EPS)
    sd = pool.tile([1, N], F32, tag="sd")
    nc.scalar.activation(sd[:], var[:], AF.Sqrt)
    rstd = pool.tile([1, N], F32, tag="rstd")
    nc.vector.reciprocal(rstd[:], sd[:])
    # physically replicate mu/rstd across partitions via a K=1 PE matmul
    # (neither DVE nor DMA can broadcast-read a single partition)
    one_row = pool.tile([1, 128], F32, tag="one_row")
    nc.vector.memset(one_row[:], 1.0)
    mu_b = pool.tile([128, N], F32, tag="mu_b")
    rstd_b = pool.tile([128, N], F32, tag="rstd_b")
    with tc.tile_pool(name="bc_psum", bufs=1, space="PSUM") as bcp:
        bc_ps = bcp.tile([128, N], F32, tag="bc_ps")
        for sl in (slice(0, 512), slice(512, N)):
            nc.tensor.matmul(bc_ps[:, sl], one_row[:], mu[:, sl],
                             start=True, stop=True)
        nc.scalar.activation(mu_b[:], bc_ps[:], AF.Copy)
        for sl in (slice(0, 512), slice(512, N)):
            nc.tensor.matmul(bc_ps[:, sl], one_row[:], rstd[:, sl],
                             start=True, stop=True)
        nc.scalar.activation(rstd_b[:], bc_ps[:], AF.Copy)

    gel = []
    for ct in range(CT):
        t1 = sq[ct]  # reuse the square tile as scratch
        nc.vector.tensor_tensor(out=t1[:], in0=off[ct][:],
                                in1=mu_b[:], op=SUB)
        nc.vector.tensor_tensor(out=t1[:], in0=t1[:],
                                in1=rstd_b[:], op=MULT)
        nc.vector.tensor_scalar(out=t1[:], in0=t1[:], scalar1=lng[ct][:, 0:1],
                                scalar2=lnb[ct][:, 0:1], op0=MULT, op1=ADD)
        g = off[ct]  # reuse the off tile for the gelu output
        nc.scalar.activation(g[:], t1[:], AF.Gelu)
        gel.append(g)

    with tc.tile_pool(name="off_psum", bufs=1, space="PSUM") as offp:
        oyx_ps = offp.tile([2, N], F32, tag="oyx")
        for sl in (slice(0, 512), slice(512, N)):
            for ct in range(CT):
                nc.tensor.matmul(oyx_ps[:, sl], w2t[ct][:], gel[ct][:, sl],
                                 start=(ct == 0), stop=(ct == CT - 1))
        pos = pool.tile([2, N], F32, tag="pos")
        nc.vector.tensor_tensor(out=pos[:], in0=oyx_ps[:], in1=refyx[:], op=ADD)
    nc.scalar.activation(pos[:], pos[:], AF.Tanh)
    ixy = pool.tile([2, N], F32, tag="ixy")
    # iy/ix = (pos + 1) * (H-1)/2
    nc.vector.tensor_scalar(out=ixy[:], in0=pos[:], scalar1=(H - 1) / 2.0,
                            scalar2=(H - 1) / 2.0, op0=MULT, op1=ADD)
    ixy_write = dma.dma_start(io["ixy_dram"][:, :], ixy[:])

    # ---------------- value conv + pixel-major table ----------------
    val = []
    for ct in range(CT):
        t = pool.tile([128, PIXPAD], F16, tag=f"val_{ct}")
        nc.vector.memset(t[:, PIX:], 0.0)
        x3 = xp16[ct][:].rearrange("p (h w) -> p h w", h=HP)
        dwconv(nc.vector, t[:, :PIX].rearrange("p (h w) -> p h w", h=H),
               x3, wv[ct], bv[ct], 0, 1, H, W)
        val.append(t)
    vtab_writes = []
    with tc.tile_pool(name="vtp", bufs=3) as vtp:
        for chunk in range(PIXPAD // 128):
            wide = vtp.tile([128, C], F16, tag="vt_wide")
            for ct in range(CT):
                dma.dma_start_transpose(wide[:, ct * 128:(ct + 1) * 128],
                                        val[ct][:, chunk * 128:(chunk + 1) * 128])
            wi = dma.dma_start(io["vtab"][chunk * 128:(chunk + 1) * 128, :], wide[:])
            vtab_writes.append(wi)

    # ---------------- key conv + transpose ----------------
    key = []
    for ct in range(CT):
        t = pool.tile([128, N], F16, tag=f"key_{ct}")
        x3 = xp16[ct][:].rearrange("p (h w) -> p h w", h=HP)
        dwconv(nc.vector, t[:].rearrange("p (h w) -> p h w", h=KH),
               x3, wk[ct], bk[ct], 0, 2, KH, KW)
        key.append(t)
    kT = []
    with tc.tile_pool(name="ktp", bufs=2, space="PSUM") as ktp:
        for k in range(NTILES):
            t = pool.tile([NT, C], F16, tag=f"kT_{k}")
            for ct in range(CT):
                ps = ktp.tile([NT, 128], F16, tag="kt_ps", space="PSUM")
                nc.tensor.transpose(ps[:], key[ct][:, k * NT:(k + 1) * NT], ident[:])
                nc.scalar.activation(t[:, ct * 128:(ct + 1) * 128], ps[:], AF.Copy)
            kT.append(t)

    # ---------------- query conv ----------------
    q16 = []
    for ct in range(CT):
        t = pool.tile([128, HALF_PIX], F16, tag=f"q_{ct}")
        x3 = xq16[ct][:].rearrange("p (h w) -> p h w", h=30)
        dwconv(nc.vector, t[:].rearrange("p (h w) -> p h w", h=HALF_ROWS),
               x3, wq[ct], bq[ct], 0, 1, HALF_ROWS, W)
        q16.append(t)

    # ---------------- indices + gathers + bilinear ----------------
    vs = []
    with tc.tile_pool(name="gat", bufs=3) as gat:
        for k in range(NTILES):
            iy_x = gat.tile([NT, 2], F32, tag="iyx")
            # partition = point, free = (y,x)
            src = bass.AP(io["ixy_dram"].tensor, k * NT, [[1, NT], [N, 2]])
            rd = dma.dma_start(iy_x[:], src)
            tile.add_dep_helper(rd.ins, ixy_write.ins, reason="ixy dram RAW")
            xy0i = gat.tile([NT, 2], I32, tag="xy0i")
            nc.vector.tensor_copy(xy0i[:], iy_x[:])
            xy0f = gat.tile([NT, 2], F32, tag="xy0f")
            nc.vector.tensor_copy(xy0f[:], xy0i[:])
            # exact floor whether the int cast truncates (sim) or rounds (hw):
            # subtract 1 wherever cast result exceeds the input
            gtm = gat.tile([NT, 2], F32, tag="gtm")
            nc.vector.tensor_tensor(out=gtm[:], in0=xy0f[:], in1=iy_x[:],
                                    op=mybir.AluOpType.is_gt)
            nc.vector.tensor_tensor(out=xy0f[:], in0=xy0f[:], in1=gtm[:], op=SUB)
            nc.vector.tensor_scalar(out=xy0f[:], in0=xy0f[:], scalar1=float(H - 2),
                                    scalar2=None, op0=mybir.AluOpType.min)
            frac = gat.tile([NT, 2], F32, tag="frac")
            nc.vector.tensor_tensor(out=frac[:], in0=iy_x[:], in1=xy0f[:], op=SUB)
            omf = gat.tile([NT, 2], F32, tag="omf")
            nc.vector.tensor_scalar(out=omf[:], in0=frac[:], scalar1=-1.0,
                                    scalar2=1.0, op0=MULT, op1=ADD)
            # bilinear weights [NT,1] each: w00=(1-wy)(1-wx), w01=(1-wy)wx,
            # w10=wy(1-wx), w11=wy*wx   (col0=y, col1=x)
            wts = gat.tile([NT, 4], F32, tag="wts")
            nc.vector.tensor_tensor(out=wts[:, 0:1], in0=omf[:, 0:1], in1=omf[:, 1:2], op=MULT)
            nc.vector.tensor_tensor(out=wts[:, 1:2], in0=omf[:, 0:1], in1=frac[:, 1:2], op=MULT)
            nc.vector.tensor_tensor(out=wts[:, 2:3], in0=frac[:, 0:1], in1=omf[:, 1:2], op=MULT)
            nc.vector.tensor_tensor(out=wts[:, 3:4], in0=frac[:, 0:1], in1=frac[:, 1:2], op=MULT)
            idxf = gat.tile([NT, 1], F32, tag="idxf")
            nc.vector.scalar_tensor_tensor(out=idxf[:], in0=xy0f[:, 0:1], scalar=float(W),
                                           in1=xy0f[:, 1:2], op0=MULT, op1=ADD)
            idxi = gat.tile([NT, 1], I32, tag="idxi")
            nc.vector.tensor_copy(idxi[:], idxf[:])

            g = [gat.tile([NT, C], F16, tag=f"g{j}", name=f"g{j}_{k}") for j in range(4)]
            for j, delta in enumerate((0, 1, W, W + 1)):
                gi = nc.gpsimd.indirect_dma_start(
                    out=g[j][:], out_offset=None, in_=io["vtab"][:, :],
                    in_offset=bass.IndirectOffsetOnAxis(ap=idxi[:, :1], axis=0),
                    element_offset=delta * C,
                    bounds_check=PIX - 1, oob_is_err=False)
                for wi in vtab_writes:
                    tile.add_dep_helper(gi.ins, wi.ins, reason="vtab RAW")
            v = pool.tile([NT, C], F16, tag=f"vs_{k}")
            nc.vector.tensor_scalar(out=v[:], in0=g[0][:], scalar1=wts[:, 0:1],
                                    scalar2=None, op0=MULT)
            for j in range(1, 4):
                nc.vector.scalar_tensor_tensor(out=v[:], in0=g[j][:], scalar=wts[:, j:j + 1],
                                               in1=v[:], op0=MULT, op1=ADD)
            vs.append(v)

    # ---------------- M = V_s K^T per head (32x32), A^T, y ----------------
    # M via independent single-shot matmuls (the sim's psum zero-region
    # tracking is partition-blind, so multi-matmul accumulation groups from
    # different heads in one bank conflict); reduce the 7 k-slots on DVE.
    m16 = []
    with tc.tile_pool(name="mps", bufs=1, space="PSUM") as mps:
        m_ps = [mps.tile([128, HC * NTILES], F32, tag=f"m_ps{i}", name=f"m_ps{i}")
                for i in range(CT)]
        for h in range(NH):
            ct, j = h // 4, h % 4
            for k in range(NTILES):
                nc.tensor.matmul(m_ps[ct][j * 32:(j + 1) * 32, k * HC:(k + 1) * HC],
                                 vs[k][:, h * HC:(h + 1) * HC],
                                 kT[k][:, h * HC:(h + 1) * HC],
                                 start=True, stop=True,
                                 tile_position=(0, j * 32))
        for ct in range(CT):
            acc = pool.tile([128, HC], F32, tag=f"m32_{ct}")
            nc.scalar.activation(acc[:], m_ps[ct][:, 0:HC], AF.Copy)
            for k in range(1, NTILES):
                nc.vector.tensor_tensor(out=acc[:], in0=acc[:],
                                        in1=m_ps[ct][:, k * HC:(k + 1) * HC], op=ADD)
            t = pool.tile([128, HC], F16, tag=f"m16_{ct}")
            nc.scalar.activation(t[:], acc[:], AF.Copy, scale=SCALE)
            m16.append(t)

    at16 = []
    with tc.tile_pool(name="atps", bufs=1, space="PSUM") as atps:
        at_ps = [atps.tile([128, C], F32, tag=f"at_ps{i}", name=f"at_ps{i}") for i in range(CT)]
        for h in range(NH):
            ct, j = h // 4, h % 4
            nc.tensor.matmul(at_ps[ct][j * 32:(j + 1) * 32, :],
                             m16[ct][j * 32:(j + 1) * 32, :],
                             wot16[ct][j * 32:(j + 1) * 32, :],
                             start=True, stop=True,
                             tile_position=(j * 32, j * 32))
        for ct in range(CT):
            t = pool.tile([128, C], F16, tag=f"at16_{ct}")
            nc.scalar.activation(t[:], at_ps[ct][:], AF.Copy)
            at16.append(t)

    NCHUNK = 4
    CW = HALF_PIX // NCHUNK  # 392
    with tc.tile_pool(name="yps", bufs=2, space="PSUM") as yps, \
         tc.tile_pool(name="ysb", bufs=3) as ysb:
        for ot in range(CT):
            for ch in range(NCHUNK):
                y_ps = yps.tile([128, CW], F32, tag="y_ps", space="PSUM")
                for ct in range(CT):
                    nc.tensor.matmul(y_ps[:], at16[ct][:, ot * 128:(ot + 1) * 128],
                                     q16[ct][:, ch * CW:(ch + 1) * CW],
                                     start=(ct == 0), stop=(ct == CT - 1))
                y_sb = ysb.tile([128, CW], F32, tag="y_sb")
                nc.scalar.activation(y_sb[:], y_ps[:], AF.Copy)
                dma.dma_start(io["y"][ot * 128:(ot + 1) * 128, ch * CW:(ch + 1) * CW],
                              y_sb[:])


def build_program():
    if "nc" in _CACHE:
        return _CACHE["nc"]
    nc = bacc.Bacc("TRN2", target_bir_lowering=False, debug=False, num_devices=8)
    io = {}
    io["xp"] = nc.dram_tensor("xp", (C, HP * HP), F32, kind="ExternalInput").ap()
    io["xq"] = nc.dram_tensor("xq", (C, 30 * HP), F32, kind="ExternalInput").ap()
    for nm, shape in [("wv", (C, 9)), ("wq", (C, 9)), ("wk", (C, 9)), ("wo", (C, 9)),
                      ("bv", (C, 1)), ("bq", (C, 1)), ("bk", (C, 1)), ("bo", (C, 1)),
                      ("lng", (C, 1)), ("lnb", (C, 1)), ("w2t", (C, 2)),
                      ("wot", (C, C)), ("refyx", (2, N))]:
        io[nm] = nc.dram_tensor(nm, shape, F32, kind="ExternalInput").ap()
    io["vtab"] = nc.dram_tensor("vtab", (PIXPAD, C), F16).ap()
    io["ixy_dram"] = nc.dram_tensor("ixy_dram", (2, N), F32).ap()
    io["y"] = nc.dram_tensor("y", (C, HALF_PIX), F32, kind="ExternalOutput").ap()

    with tile.TileContext(nc) as tc:
        with contextlib.ExitStack() as ctx:
            _emit(nc, tc, ctx, io)
    nc.compile()
    _CACHE["nc"] = nc
    return nc


def host_prep(inputs):
    """Build the 8 per-core input maps from full inputs."""
    x = np.asarray(inputs["x"], np.float32)          # (B, C, H, W)
    xpad = np.pad(x, ((0, 0), (0, 0), (1, 1), (1, 1)))  # (B, C, 58, 58)
    shared = {}
    for nm, src in [("wv", "w_v"), ("wq", "w_q"), ("wk", "w_k"), ("wo", "w_off1")]:
        shared[nm] = np.asarray(inputs[src], np.float32).reshape(C, 9)
    for nm, src in [("bv", "b_v"), ("bq", "b_q"), ("bk", "b_k"), ("bo", "b_off1"),
                    ("lng", "ln_g"), ("lnb", "ln_b")]:
        shared[nm] = np.asarray(inputs[src], np.float32).reshape(C, 1)
    shared["w2t"] = np.ascontiguousarray(np.asarray(inputs["w_off2"], np.float32).T)  # (C,2)
    shared["wot"] = np.ascontiguousarray(np.asarray(inputs["w_out"], np.float32).T)   # (C,C) [c,o]
    ry = (np.arange(KH, dtype=np.float32) + 0.5) / KH * 2 - 1
    rx = (np.arange(KW, dtype=np.float32) + 0.5) / KW * 2 - 1
    refyx = np.stack([np.repeat(ry, KW), np.tile(rx, KH)])   # (2, 784), row0=y
    shared["refyx"] = np.ascontiguousarray(refyx, dtype=np.float32)

    in_maps = []
    for core in range(8):
        b, half = core // 2, core % 2
        m = dict(shared)
        m["xp"] = np.ascontiguousarray(xpad[b].reshape(C, HP * HP))
        r0 = half * HALF_ROWS
        m["xq"] = np.ascontiguousarray(xpad[b, :, r0:r0 + 30, :].reshape(C, 30 * HP))
        in_maps.append(m)
    return in_maps


def assemble(results):
    y = np.empty((B, C, H, W), np.float32)
    for core in range(8):
        b, half = core // 2, core % 2
        part = results[core]["y"].reshape(C, HALF_ROWS, W)
        y[b, :, half * HALF_ROWS:(half + 1) * HALF_ROWS, :] = part
    return y


def run(inputs, trace=False):
    nc = build_program()
    in_maps = host_prep(inputs)
    res = run_bass_kernel_spmd(nc, in_maps, core_ids=list(range(8)), trace=trace)
    return assemble(res.results), res


def kernel(**inputs):
    out, _ = run(inputs, trace=False)
    return out



# revision 19
# speedup vs baseline: 1.9838x; 1.9838x over previous
"""Trainium2 Bass kernel for nn_DeformableAttention (B=4, C=384, H=W=56, NH=12, HC=32, STRIDE=2).

Self-contained: hardcodes shapes/sharding. Sharding: 8 cores = 4 batches x 2
pixel-row-halves. Each core computes the full value/key/offset branches for its
batch (duplicated across the pair) and the query branch + final GEMM for its
half of the 3136 output pixels.

Math note: the reference computes out = (scale * q^T k) v^T without softmax, so
attention is linear and reassociates:
    y[b] = (w_out @ blockdiag_h(scale * M[b,h])) @ Q[b],
    M[b,h] = V_s[b,h] K[b,h]^T  (32x32 per head)
which drops the 48x(3136x784x32) einsums to a few small GEMMs.

Measured HW model (trace-derived): DVE ~1.1ns/elem regardless of dtype/stride;
PE matmul ~M+128 cycles; Pool elementwise ~2ns/elem with a one-time ~60us
ucode-load on the first op (pre-warmed with dummies); per-DMA-queue bandwidth
~90GB/s (big loads are split into slices to ride multiple queues).

Schedule: value conv runs on the PE as 9 diag(w_tap) matmuls PSUM-accumulated
per 448-pixel chunk (bias folded into the PSUM->SBUF copy), freeing ~92us of
DVE. The off branch is fp32 end-to-end (sample positions are precision
critical). DVE keeps off/key/query convs + LN pointwise + bilinear. Gathers
fetch (x0,x0+1) pixel pairs as one 768-elem row. M accumulates over the 7
k-tiles directly in PSUM. floor() is computed via round(x-0.5) (casts round to
nearest; integer ties land on the complementary-weight corner, which is exact).
"""
import contextlib

import numpy as np

import concourse.bass as bass
import concourse.tile as tile
from concourse import bacc, mybir
from concourse.bass_utils import run_bass_kernel_spmd
from concourse.masks import make_identity

F32, F16, I32 = mybir.dt.float32, mybir.dt.float16, mybir.dt.int32
MULT, ADD, SUB = mybir.AluOpType.mult, mybir.AluOpType.add, mybir.AluOpType.subtract
AF = mybir.ActivationFunctionType

B, C, H, W = 4, 384, 56, 56
NH, HC = 12, 32
SCALE = HC ** -0.5
HP = H + 2                      # 58 padded
PIX = H * W                     # 3136
KH = KW = 28                    # stride-2 output
N = KH * KW                     # 784
NT = 112                        # point-tile size (7 tiles)
NTILES = N // NT
HALF_ROWS = H // 2              # 28
HALF_PIX = HALF_ROWS * W        # 1568
CT = C // 128                   # 3 channel tiles
EPS = 1e-5
VCH = 448                       # value-conv PE chunk (8 rows of 56)
NVCH = PIX // VCH               # 7 chunks per ct

_CACHE = {}


def _emit(nc, tc, ctx, io):
    pool = ctx.enter_context(tc.tile_pool(name="main", bufs=1))
    dma = nc.sync
    dma2 = nc.scalar
    gp = nc.gpsimd

    # ---------------- loads (big tensors split across DMA queues) ----------------
    def load_split(name, width, dtype, nsplit, eng):
        out = []
        for ct in range(CT):
            t = pool.tile([128, width], dtype, tag=f"{name}_{ct}")
            step = 128 // nsplit
            for s in range(nsplit):
                r0 = s * step
                eng.dma_start(t[r0:r0 + step, :],
                              io[name][ct * 128 + r0:ct * 128 + r0 + step, :])
            out.append(t)
        return out

    def load_cols(name, width, dtype=F32, eng=dma):
        out = []
        for ct in range(CT):
            t = pool.tile([128, width], dtype, tag=f"{name}_{ct}")
            eng.dma_start(t[:], io[name][ct * 128:(ct + 1) * 128, :])
            out.append(t)
        return out

    wv = load_cols("wv", 9)
    bv = load_cols("bv", 1)
    wo = load_cols("wo", 9)
    bo = load_cols("bo", 1)
    xp16 = load_split("xp16", HP * HP, F16, 2, dma2)
    xp32 = load_split("xp32", HP * HP, F32, 4, dma)
    wk = load_cols("wk", 9)
    bk = load_cols("bk", 1)
    xq16 = load_split("xq", 30 * HP, F16, 2, dma2)
    wq = load_cols("wq", 9)
    bq = load_cols("bq", 1)
    lng = load_cols("lng", 1)
    lnb = load_cols("lnb", 1)
    w2t32 = load_cols("w2t", 2)
    wot16 = load_cols("wot", C, dtype=F16, eng=dma2)
    refyx = pool.tile([2, N], F32, tag="refyx")
    dma.dma_start(refyx[:], io["refyx"][:, :])
    ones_rc = pool.tile([128, 1], F16, tag="ones_rc")
    nc.vector.memset(ones_rc[:], 1.0 / C)
    one_row = pool.tile([1, 128], F16, tag="one_row")
    nc.vector.memset(one_row[:], 1.0)
    ident = pool.tile([128, 128], F16, tag="ident")
    make_identity(nc, ident[:])
    eps_t = pool.tile([1, 1], F32, tag="eps_t")
    nc.vector.memset(eps_t[:], EPS)

    # pre-warm Pool elementwise ucode (first op pays ~60us load otherwise)
    wrm = pool.tile([2, 2], F32, tag="wrm")
    wrmi = pool.tile([2, 2], I32, tag="wrmi")
    nc.vector.memset(wrm[:], 1.0)
    gp.tensor_scalar(out=wrm[:], in0=wrm[:], scalar1=1.0, scalar2=0.5,
                     op0=MULT, op1=ADD)
    gp.tensor_tensor(out=wrm[:], in0=wrm[:], in1=wrm[:], op=SUB)
    gp.tensor_tensor(out=wrm[:], in0=wrm[:], in1=wrm[:], op=MULT)
    gp.tensor_tensor(out=wrm[:], in0=wrm[:], in1=wrm[:], op=ADD)
    gp.tensor_copy(wrmi[:], wrm[:])
    gp.tensor_copy(wrm[:], wrmi[:])
    gp.tensor_scalar(out=wrm[:], in0=wrm[:], scalar1=54.0, scalar2=0.0,
                     op0=mybir.AluOpType.min, op1=mybir.AluOpType.max)

    # ---------------- diag weight tiles for the PE value conv ----------------
    diag = []
    for ct in range(CT):
        dd = []
        for t in range(9):
            d = pool.tile([128, 128], F16, tag=f"diag_{ct}_{t}")
            nc.vector.tensor_scalar(out=d[:], in0=ident[:], scalar1=wv[ct][:, t:t + 1],
                                    scalar2=None, op0=MULT)
            dd.append(d)
        diag.append(dd)

    # ---------------- strided dwconv helper (DVE) ----------------
    def dwconv(out2, xt, w, b, stride, rows, cols):
        for t in range(9):
            dy, dx = t // 3, t % 3
            src = xt[:, dy:dy + (rows - 1) * stride + 1:stride,
                     dx:dx + (cols - 1) * stride + 1:stride]
            if t == 0:
                nc.vector.tensor_scalar(out=out2, in0=src, scalar1=w[:, 0:1],
                                        scalar2=b[:, 0:1], op0=MULT, op1=ADD)
            else:
                nc.vector.scalar_tensor_tensor(out=out2, in0=src, scalar=w[:, t:t + 1],
                                               in1=out2, op0=MULT, op1=ADD)

    # ---------------- off conv (DVE, fp32) ----------------
    off = []
    for ct in range(CT):
        t = pool.tile([128, N], F32, tag=f"off_{ct}")
        x3 = xp32[ct][:].rearrange("p (h w) -> p h w", h=HP)
        dwconv(t[:].rearrange("p (h w) -> p h w", h=KH), x3, wo[ct], bo[ct],
               2, KH, KW)
        off.append(t)

    # f16 copies for the LN-stats matmuls (stats precision is not critical)
    off16, sq16 = [], []
    for ct in range(CT):
        t = pool.tile([128, N], F16, tag=f"off16_{ct}")
        nc.scalar.activation(t[:], off[ct][:], AF.Copy)
        off16.append(t)
        s = pool.tile([128, N], F16, tag=f"sq16_{ct}")
        nc.scalar.activation(s[:], off[ct][:], AF.Square)
        sq16.append(s)

    # ---------------- value conv ct0 (PE diag-matmul, PSUM-accumulated) ------
    val = [pool.tile([128, PIX], F16, tag=f"val_{ct}", name=f"val_{ct}")
           for ct in range(CT)]
    vps_ctx = tc.tile_pool(name="vps", bufs=2, space="PSUM")
    vps = vps_ctx.__enter__()

    def value_ct(ct):
        x3 = xp16[ct][:].rearrange("p (h w) -> p h w", h=HP)
        for chk in range(NVCH):
            r0 = chk * 8  # output row base of this 448-pixel chunk
            ps = vps.tile([128, VCH], F32, tag="vch", space="PSUM")
            for t in range(9):
                dy, dx = t // 3, t % 3
                src = x3[:, r0 + dy:r0 + dy + 8, dx:dx + W]
                nc.tensor.matmul(ps[:].rearrange("p (h w) -> p h w", h=8),
                                 diag[ct][t], src,
                                 start=(t == 0), stop=(t == 8),
                                 skip_group_check=True)
            nc.scalar.activation(val[ct][:, chk * VCH:(chk + 1) * VCH], ps[:],
                                 AF.Identity, bias=bv[ct][:, 0:1])

    value_ct(0)

    # ---------------- vtab band 0 (PE transposes, 4-chunk groups) ----------------
    vtab_writes = []
    vtctx = tc.tile_pool(name="vtp_ps", bufs=2, space="PSUM")
    vtps = vtctx.__enter__()
    vtsctx = tc.tile_pool(name="vtp_sb", bufs=3)
    vtsb = vtsctx.__enter__()

    def vtab_band(ct):
        for g in range(6):
            c0 = g * 512
            tp4 = vtps.tile([128, 512], F16, tag="tp4", space="PSUM")
            for j in range(4):
                nc.tensor.transpose(tp4[:, j * 128:(j + 1) * 128],
                                    val[ct][:, c0 + j * 128:c0 + (j + 1) * 128],
                                    ident[:])
            w4 = vtsb.tile([128, 512], F16, tag="w4")
            nc.scalar.activation(w4[:], tp4[:], AF.Copy)
            out_ap = bass.AP(io["vtab"].tensor, c0 * C + ct * 128,
                             [[C, 128], [C * 128, 4], [1, 128]])
            vtab_writes.append(dma.dma_start(out_ap, w4[:]))
        tp1 = vtps.tile([64, 128], F16, tag="tp1", space="PSUM")
        nc.tensor.transpose(tp1[:], val[ct][:, 3072:3136], ident[:])
        w1 = vtsb.tile([64, 128], F16, tag="w1")
        nc.scalar.activation(w1[:], tp1[:], AF.Copy)
        vtab_writes.append(
            dma.dma_start(io["vtab"][3072:3136, ct * 128:(ct + 1) * 128], w1[:]))

    vtab_band(0)

    # ---------------- LN stats (PE f16) + pointwise (DVE) ----------------
    musq = pool.tile([1, N], F32, tag="musq")
    var = pool.tile([1, N], F32, tag="var")
    sd32 = pool.tile([1, N], F32, tag="sd32")
    rstd32 = pool.tile([1, N], F32, tag="rstd32")
    mu16 = pool.tile([1, N], F16, tag="mu16")
    rstd16 = pool.tile([1, N], F16, tag="rstd16")
    mu_b = pool.tile([128, N], F32, tag="mu_b")
    rstd_b = pool.tile([128, N], F32, tag="rstd_b")
    with tc.tile_pool(name="ln_psum", bufs=1, space="PSUM") as lnp:
        st_ps = lnp.tile([1, N], F32, tag="st_ps")
        for sl in (slice(0, 512), slice(512, N)):
            for ct in range(CT):
                nc.tensor.matmul(st_ps[:, sl], ones_rc[:], off16[ct][:, sl],
                                 start=(ct == 0), stop=(ct == CT - 1))
        nc.scalar.activation(musq[:], st_ps[:], AF.Square)
        nc.scalar.activation(mu16[:], st_ps[:], AF.Copy)
        for sl in (slice(0, 512), slice(512, N)):
            for ct in range(CT):
                nc.tensor.matmul(st_ps[:, sl], ones_rc[:], sq16[ct][:, sl],
                                 start=(ct == 0), stop=(ct == CT - 1))
        nc.vector.tensor_tensor(out=var[:], in0=st_ps[:], in1=musq[:], op=SUB)
    nc.scalar.activation(sd32[:], var[:], AF.Sqrt, bias=eps_t[:, 0:1])
    nc.vector.reciprocal_approx_fast(rstd32[:], sd32[:])
    nc.scalar.activation(rstd16[:], rstd32[:], AF.Copy)
    with tc.tile_pool(name="bc_psum", bufs=1, space="PSUM") as bcp:
        bc_ps = bcp.tile([128, N], F32, tag="bc_ps")
        for sl in (slice(0, 512), slice(512, N)):
            nc.tensor.matmul(bc_ps[:, sl], one_row[:], mu16[:, sl],
                             start=True, stop=True)
        nc.scalar.activation(mu_b[:], bc_ps[:], AF.Copy)
        for sl in (slice(0, 512), slice(512, N)):
            nc.tensor.matmul(bc_ps[:, sl], one_row[:], rstd16[:, sl],
                             start=True, stop=True)
        nc.scalar.activation(rstd_b[:], bc_ps[:], AF.Copy)

    # normalize (DVE, fp32) + gelu (scalar, g/b folded into activation)
    gel = []
    for ct in range(CT):
        t1 = off[ct]  # in-place
        nc.vector.tensor_tensor(out=t1[:], in0=t1[:], in1=mu_b[:], op=SUB)
        nc.vector.tensor_tensor(out=t1[:], in0=t1[:], in1=rstd_b[:], op=MULT)
        g = pool.tile([128, N], F32, tag=f"gel_{ct}")
        nc.scalar.activation(g[:], t1[:], AF.Gelu,
                             scale=lng[ct][:, 0:1], bias=lnb[ct][:, 0:1])
        gel.append(g)

    # ---------------- value ct1 + band1 ----------------
    value_ct(1)
    vtab_band(1)

    # ---------------- offset head: w2t matmul (fp32) + tanh + ixy ----------------
    pos = pool.tile([2, N], F32, tag="pos")
    tnh = pool.tile([2, N], F32, tag="tnh")
    ixy0 = pool.tile([2, N], F32, tag="ixy0")
    with tc.tile_pool(name="off_psum", bufs=1, space="PSUM") as offp:
        oyx_ps = offp.tile([2, N], F32, tag="oyx")
        for sl in (slice(0, 512), slice(512, N)):
            for ct in range(CT):
                nc.tensor.matmul(oyx_ps[:, sl], w2t32[ct][:], gel[ct][:, sl],
                                 start=(ct == 0), stop=(ct == CT - 1))
        oyx_sb = pool.tile([2, N], F32, tag="oyx_sb")
        nc.scalar.activation(oyx_sb[:], oyx_ps[:], AF.Copy)
    gp.tensor_tensor(out=pos[:], in0=oyx_sb[:], in1=refyx[:], op=ADD)
    nc.scalar.activation(tnh[:], pos[:], AF.Tanh)
    # iy/ix - 0.5 = tanh*27.5 + 27.0  (the -0.5 shift makes round() act as floor)
    gp.tensor_scalar(out=ixy0[:], in0=tnh[:], scalar1=(H - 1) / 2.0,
                     scalar2=(H - 1) / 2.0 - 0.5, op0=MULT, op1=ADD)
    ixy_write = dma.dma_start(io["ixy_dram"][:, :], ixy0[:])

    # ---------------- value ct2 + band2 ----------------
    value_ct(2)
    vtab_band(2)
    vtsctx.__exit__(None, None, None)
    vtctx.__exit__(None, None, None)
    vps_ctx.__exit__(None, None, None)

    # ---------------- key conv (DVE) + kT (PE transpose) ----------------
    key = []
    for ct in range(CT):
        t = pool.tile([128, N], F16, tag=f"key_{ct}")
        x3 = xp32[ct][:].rearrange("p (h w) -> p h w", h=HP)
        dwconv(t[:].rearrange("p (h w) -> p h w", h=KH), x3, wk[ct], bk[ct],
               2, KH, KW)
        key.append(t)
    kT = []
    with tc.tile_pool(name="ktp", bufs=3, space="PSUM") as ktp:
        for k in range(NTILES):
            t = pool.tile([NT, C], F16, tag=f"kT_{k}")
            for ct in range(CT):
                ps = ktp.tile([NT, 128], F16, tag="kt_ps", space="PSUM")
                nc.tensor.transpose(ps[:], key[ct][:, k * NT:(k + 1) * NT], ident[:])
                nc.scalar.activation(t[:, ct * 128:(ct + 1) * 128], ps[:], AF.Copy)
            kT.append(t)

    # ---------------- query conv (DVE) ----------------
    q16 = []
    for ct in range(CT):
        t = pool.tile([128, HALF_PIX], F16, tag=f"q_{ct}")
        x3 = xq16[ct][:].rearrange("p (h w) -> p h w", h=30)
        dwconv(t[:].rearrange("p (h w) -> p h w", h=HALF_ROWS), x3,
               wq[ct], bq[ct], 1, HALF_ROWS, W)
        q16.append(t)

    # ---------------- index math (gpsimd, wide tiles) ----------------
    # layout [112 pts, 14]: cols 0..6 = iy-0.5 per k-tile, cols 7..13 = ix-0.5
    iyx = pool.tile([NT, 2 * NTILES], F32, tag="iyx")
    for j in range(2):
        src = bass.AP(io["ixy_dram"].tensor, j * N, [[1, NT], [NT, NTILES]])
        rd = dma.dma_start(iyx[:, j * NTILES:(j + 1) * NTILES], src)
        tile.add_dep_helper(rd.ins, ixy_write.ins, reason="ixy dram RAW")
    x0i = pool.tile([NT, 2 * NTILES], I32, tag="x0i")
    gp.tensor_copy(x0i[:], iyx[:])          # round(v-0.5) == floor(v)
    x0f = pool.tile([NT, 2 * NTILES], F32, tag="x0f")
    gp.tensor_copy(x0f[:], x0i[:])
    gp.tensor_scalar(out=x0f[:], in0=x0f[:], scalar1=float(H - 2), scalar2=0.0,
                     op0=mybir.AluOpType.min, op1=mybir.AluOpType.max)
    frac = pool.tile([NT, 2 * NTILES], F32, tag="frac")
    gp.tensor_tensor(out=frac[:], in0=iyx[:], in1=x0f[:], op=SUB)
    gp.tensor_scalar(out=frac[:], in0=frac[:], scalar1=1.0, scalar2=0.5,
                     op0=MULT, op1=ADD)
    omf = pool.tile([NT, 2 * NTILES], F32, tag="omf")
    gp.tensor_scalar(out=omf[:], in0=frac[:], scalar1=-1.0, scalar2=1.0,
                     op0=MULT, op1=ADD)
    ys, xs = slice(0, NTILES), slice(NTILES, 2 * NTILES)
    wts = [pool.tile([NT, NTILES], F32, tag=f"wts{j}", name=f"wts{j}") for j in range(4)]
    gp.tensor_tensor(out=wts[0][:], in0=omf[:, ys], in1=omf[:, xs], op=MULT)
    gp.tensor_tensor(out=wts[1][:], in0=omf[:, ys], in1=frac[:, xs], op=MULT)
    gp.tensor_tensor(out=wts[2][:], in0=frac[:, ys], in1=omf[:, xs], op=MULT)
    gp.tensor_tensor(out=wts[3][:], in0=frac[:, ys], in1=frac[:, xs], op=MULT)
    idxf = pool.tile([NT, NTILES], F32, tag="idxf")
    gp.tensor_scalar(out=idxf[:], in0=x0f[:, ys], scalar1=float(W),
                     scalar2=None, op0=MULT)
    gp.tensor_tensor(out=idxf[:], in0=idxf[:], in1=x0f[:, xs], op=ADD)
    idxi = pool.tile([NT, NTILES], I32, tag="idxi")
    gp.tensor_copy(idxi[:], idxf[:])

    # ---------------- gathers + bilinear + M (PSUM-accumulated) ----------------
    vs = []
    with tc.tile_pool(name="m_psum", bufs=1, space="PSUM") as mps, \
         tc.tile_pool(name="gat", bufs=3) as gat:
        m_ps = [mps.tile([128, HC], F32, tag=f"m_ps{i}", name=f"m_ps{i}")
                for i in range(CT)]
        for k in range(NTILES):
            g0 = gat.tile([NT, 2 * C], F16, tag="g0")
            g1 = gat.tile([NT, 2 * C], F16, tag="g1")
            for g, delta in ((g0, 0), (g1, W)):
                gi = gp.indirect_dma_start(
                    out=g[:], out_offset=None, in_=io["vtab"][:, :],
                    in_offset=bass.IndirectOffsetOnAxis(ap=idxi[:, k:k + 1], axis=0),
                    element_offset=delta * C,
                    bounds_check=PIX - 1, oob_is_err=False)
                for wi in vtab_writes:
                    tile.add_dep_helper(gi.ins, wi.ins, reason="vtab RAW")
            v = pool.tile([NT, C], F16, tag=f"vs_{k}")
            nc.vector.tensor_scalar(out=v[:], in0=g0[:, 0:C], scalar1=wts[0][:, k:k + 1],
                                    scalar2=None, op0=MULT)
            nc.vector.scalar_tensor_tensor(out=v[:], in0=g0[:, C:2 * C],
                                           scalar=wts[1][:, k:k + 1], in1=v[:],
                                           op0=MULT, op1=ADD)
            nc.vector.scalar_tensor_tensor(out=v[:], in0=g1[:, 0:C],
                                           scalar=wts[2][:, k:k + 1], in1=v[:],
                                           op0=MULT, op1=ADD)
            nc.vector.scalar_tensor_tensor(out=v[:], in0=g1[:, C:2 * C],
                                           scalar=wts[3][:, k:k + 1], in1=v[:],
                                           op0=MULT, op1=ADD)
            vs.append(v)
            for h in range(NH):
                ct, j = h // 4, h % 4
                nc.tensor.matmul(m_ps[ct][j * 32:(j + 1) * 32, :],
                                 v[:, h * HC:(h + 1) * HC],
                                 kT[k][:, h * HC:(h + 1) * HC],
                                 start=(k == 0), stop=(k == NTILES - 1),
                                 tile_position=(0, j * 32),
                                 skip_group_check=True)
        m16 = []
        for ct in range(CT):
            t = pool.tile([128, HC], F16, tag=f"m16_{ct}")
            nc.scalar.activation(t[:], m_ps[ct][:], AF.Copy, scale=SCALE)
            m16.append(t)

    # ---------------- A^T = blockdiag(scale*M)^T w_out^T, then y ----------------
    at16 = []
    with tc.tile_pool(name="atps", bufs=1, space="PSUM") as atps:
        at_ps = [atps.tile([128, C], F32, tag=f"at_ps{i}", name=f"at_ps{i}")
                 for i in range(CT)]
        for h in range(NH):
            ct, j = h // 4, h % 4
            nc.tensor.matmul(at_ps[ct][j * 32:(j + 1) * 32, :],
                             m16[ct][j * 32:(j + 1) * 32, :],
                             wot16[ct][j * 32:(j + 1) * 32, :],
                             start=True, stop=True,
                             tile_position=(j * 32, j * 32))
        for ct in range(CT):
            t = pool.tile([128, C], F16, tag=f"at16_{ct}")
            nc.scalar.activation(t[:], at_ps[ct][:], AF.Copy)
            at16.append(t)

    NCHUNK = 4
    CW = HALF_PIX // NCHUNK  # 392
    with tc.tile_pool(name="yps", bufs=2, space="PSUM") as yps, \
         tc.tile_pool(name="ysb", bufs=3) as ysb:
        for ot in range(CT):
            for ch in range(NCHUNK):
                y_ps = yps.tile([128, CW], F32, tag="y_ps", space="PSUM")
                for ct in range(CT):
                    nc.tensor.matmul(y_ps[:], at16[ct][:, ot * 128:(ot + 1) * 128],
                                     q16[ct][:, ch * CW:(ch + 1) * CW],
                                     start=(ct == 0), stop=(ct == CT - 1))
                y_sb = ysb.tile([128, CW], F32, tag="y_sb")
                nc.scalar.activation(y_sb[:], y_ps[:], AF.Copy)
                dma.dma_start(io["y"][ot * 128:(ot + 1) * 128, ch * CW:(ch + 1) * CW],
                              y_sb[:])


def build_program():
    if "nc" in _CACHE:
        return _CACHE["nc"]
    nc = bacc.Bacc("TRN2", target_bir_lowering=False, debug=False, num_devices=8)
    io = {}
    io["xp16"] = nc.dram_tensor("xp16", (C, HP * HP), F16, kind="ExternalInput").ap()
    io["xp32"] = nc.dram_tensor("xp32", (C, HP * HP), F32, kind="ExternalInput").ap()
    io["xq"] = nc.dram_tensor("xq", (C, 30 * HP), F16, kind="ExternalInput").ap()
    for nm, shape in [("wv", (C, 9)), ("wq", (C, 9)), ("wk", (C, 9)), ("wo", (C, 9)),
                      ("bv", (C, 1)), ("bq", (C, 1)), ("bk", (C, 1)), ("bo", (C, 1)),
                      ("lng", (C, 1)), ("lnb", (C, 1)), ("w2t", (C, 2)),
                      ("refyx", (2, N))]:
        io[nm] = nc.dram_tensor(nm, shape, F32, kind="ExternalInput").ap()
    io["wot"] = nc.dram_tensor("wot", (C, C), F16, kind="ExternalInput").ap()
    io["vtab"] = nc.dram_tensor("vtab", (PIX, C), F16).ap()
    io["ixy_dram"] = nc.dram_tensor("ixy_dram", (2, N), F32).ap()
    io["y"] = nc.dram_tensor("y", (C, HALF_PIX), F32, kind="ExternalOutput").ap()

    with tile.TileContext(nc) as tc:
        with contextlib.ExitStack() as ctx:
            _emit(nc, tc, ctx, io)
    nc.compile()
    _CACHE["nc"] = nc
    return nc


def host_prep(inputs):
    """Build the 8 per-core input maps from full inputs."""
    x = np.asarray(inputs["x"], np.float32)          # (B, C, H, W)
    xpad = np.pad(x, ((0, 0), (0, 0), (1, 1), (1, 1)))  # (B, C, 58, 58)
    shared = {}
    for nm, src in [("wv", "w_v"), ("wq", "w_q"), ("wk", "w_k"), ("wo", "w_off1")]:
        shared[nm] = np.asarray(inputs[src], np.float32).reshape(C, 9)
    for nm, src in [("bv", "b_v"), ("bq", "b_q"), ("bk", "b_k"), ("bo", "b_off1"),
                    ("lng", "ln_g"), ("lnb", "ln_b")]:
        shared[nm] = np.asarray(inputs[src], np.float32).reshape(C, 1)
    shared["w2t"] = np.ascontiguousarray(np.asarray(inputs["w_off2"], np.float32).T)
    shared["wot"] = np.ascontiguousarray(
        np.asarray(inputs["w_out"], np.float32).T).astype(np.float16)   # (C,C) [c,o]
    ry = (np.arange(KH, dtype=np.float32) + 0.5) / KH * 2 - 1
    rx = (np.arange(KW, dtype=np.float32) + 0.5) / KW * 2 - 1
    refyx = np.stack([np.repeat(ry, KW), np.tile(rx, KH)])   # (2, 784), row0=y
    shared["refyx"] = np.ascontiguousarray(refyx, dtype=np.float32)

    in_maps = []
    for core in range(8):
        b, half = core // 2, core % 2
        m = dict(shared)
        xb = xpad[b]
        m["xp32"] = np.ascontiguousarray(xb.reshape(C, HP * HP))
        m["xp16"] = m["xp32"].astype(np.float16)
        r0 = half * HALF_ROWS
        m["xq"] = np.ascontiguousarray(
            xb[:, r0:r0 + 30, :].reshape(C, 30 * HP)).astype(np.float16)
        in_maps.append(m)
    return in_maps


def assemble(results):
    y = np.empty((B, C, H, W), np.float32)
    for core in range(8):
        b, half = core // 2, core % 2
        part = results[core]["y"].reshape(C, HALF_ROWS, W)
        y[b, :, half * HALF_ROWS:(half + 1) * HALF_ROWS, :] = part
    return y


def run(inputs, trace=False):
    nc = build_program()
    in_maps = host_prep(inputs)
    res = run_bass_kernel_spmd(nc, in_maps, core_ids=list(range(8)), trace=trace)
    return assemble(res.results), res


def kernel(**inputs):
    out, _ = run(inputs, trace=False)
    return out


# revision 21
# speedup vs baseline: 2.1468x; 1.0822x over previous
"""Trainium2 Bass kernel for nn_DeformableAttention (B=4, C=384, H=W=56, NH=12, HC=32, STRIDE=2).

Self-contained: hardcodes shapes/sharding. Sharding: 8 cores = 4 batches x 2
pixel-row-halves. Each core computes the full value/key/offset branches for its
batch (duplicated across the pair) and the query branch + final GEMM for its
half of the 3136 output pixels.

Math note: the reference computes out = (scale * q^T k) v^T without softmax, so
attention is linear and reassociates:
    y[b] = (w_out @ blockdiag_h(scale * M[b,h])) @ Q[b],
    M[b,h] = V_s[b,h] K[b,h]^T  (32x32 per head)
which drops the 48x(3136x784x32) einsums to a few small GEMMs.

Measured HW model (trace-derived): DVE ~1.1ns/elem regardless of dtype/stride;
PE matmul ~M+128 cycles; Pool elementwise ~2ns/elem with a one-time ~60us
ucode-load on the first op (pre-warmed with dummies); per-DMA-queue bandwidth
~90GB/s (big loads are split into slices to ride multiple queues).

Schedule: value conv runs on the PE as 9 diag(w_tap) matmuls PSUM-accumulated
per 448-pixel chunk (bias folded into the PSUM->SBUF copy), freeing ~92us of
DVE. The off branch is fp32 end-to-end (sample positions are precision
critical). DVE keeps off/key/query convs + LN pointwise + bilinear. Gathers
fetch (x0,x0+1) pixel pairs as one 768-elem row. M accumulates over the 7
k-tiles directly in PSUM. floor() is computed via round(x-0.5) (casts round to
nearest; integer ties land on the complementary-weight corner, which is exact).
"""
import contextlib

import numpy as np

import concourse.bass as bass
import concourse.tile as tile
from concourse import bacc, mybir
from concourse.bass_utils import run_bass_kernel_spmd
from concourse.masks import make_identity

F32, F16, I32 = mybir.dt.float32, mybir.dt.float16, mybir.dt.int32
MULT, ADD, SUB = mybir.AluOpType.mult, mybir.AluOpType.add, mybir.AluOpType.subtract
AF = mybir.ActivationFunctionType

B, C, H, W = 4, 384, 56, 56
NH, HC = 12, 32
SCALE = HC ** -0.5
HP = H + 2                      # 58 padded
PIX = H * W                     # 3136
KH = KW = 28                    # stride-2 output
N = KH * KW                     # 784
NT = 112                        # point-tile size (7 tiles)
NTILES = N // NT
HALF_ROWS = H // 2              # 28
HALF_PIX = HALF_ROWS * W        # 1568
CT = C // 128                   # 3 channel tiles
EPS = 1e-5
VCH = 448                       # value-conv PE chunk (8 rows of 56)
NVCH = PIX // VCH               # 7 chunks per ct

_CACHE = {}


def _emit(nc, tc, ctx, io):
    pool = ctx.enter_context(tc.tile_pool(name="main", bufs=1))
    dma = nc.sync
    gp = nc.gpsimd

    # ---------------- loads (big tensors split across DMA queues) ----------------
    def load_split(name, width, dtype, nsplit, eng):
        out = []
        for ct in range(CT):
            t = pool.tile([128, width], dtype, tag=f"{name}_{ct}")
            step = 128 // nsplit
            for s in range(nsplit):
                r0 = s * step
                eng.dma_start(t[r0:r0 + step, :],
                              io[name][ct * 128 + r0:ct * 128 + r0 + step, :])
            out.append(t)
        return out

    def load_cols(name, width, dtype=F32, eng=dma):
        out = []
        for ct in range(CT):
            t = pool.tile([128, width], dtype, tag=f"{name}_{ct}")
            eng.dma_start(t[:], io[name][ct * 128:(ct + 1) * 128, :])
            out.append(t)
        return out

    wv = load_cols("wv", 9)
    bv = load_cols("bv", 1)
    wo = load_cols("wo", 9)
    bo = load_cols("bo", 1)
    xp16 = load_split("xp16", HP * HP, F16, 4, dma)
    wk = load_cols("wk", 9)
    bk = load_cols("bk", 1)
    xq16 = load_split("xq", 30 * HP, F16, 2, dma)
    wq = load_cols("wq", 9)
    bq = load_cols("bq", 1)
    lng = load_cols("lng", 1)
    lnb = load_cols("lnb", 1)
    w2t32 = load_cols("w2t", 2)
    wot16 = load_cols("wot", C, dtype=F16, eng=dma)
    refyx = pool.tile([2, N], F32, tag="refyx")
    dma.dma_start(refyx[:], io["refyx"][:, :])
    ones_rc = pool.tile([128, 1], F16, tag="ones_rc")
    nc.vector.memset(ones_rc[:], 1.0 / C)
    one_row = pool.tile([1, 128], F16, tag="one_row")
    nc.vector.memset(one_row[:], 1.0)
    ident = pool.tile([128, 128], F16, tag="ident")
    make_identity(nc, ident[:])
    eps_t = pool.tile([1, 1], F32, tag="eps_t")
    nc.vector.memset(eps_t[:], EPS)

    # diag(w_tap) tiles (scalar engine: per-partition scale of the identity)
    def make_diag(w, nm):
        out = []
        for ct in range(CT):
            dd = []
            for t in range(9):
                d = pool.tile([128, 128], F16, tag=f"dg_{nm}_{ct}_{t}",
                              name=f"dg_{nm}_{ct}_{t}")
                nc.scalar.activation(d[:], ident[:], AF.Copy, scale=w[ct][:, t:t + 1])
                dd.append(d)
            out.append(dd)
        return out

    diag_v = make_diag(wv, "v")

    # ---------------- off conv (DVE, fp32 accumulate) ----------------
    off = []
    for ct in range(CT):
        t = pool.tile([128, N], F32, tag=f"off_{ct}")
        x3 = xp16[ct][:].rearrange("p (h w) -> p h w", h=HP)
        for tap in range(9):
            dy, dx = tap // 3, tap % 3
            src = x3[:, dy:dy + 2 * KH - 1:2, dx:dx + 2 * KW - 1:2]
            if tap == 0:
                nc.vector.tensor_scalar(out=t[:].rearrange("p (h w) -> p h w", h=KH),
                                        in0=src, scalar1=wo[ct][:, 0:1],
                                        scalar2=bo[ct][:, 0:1], op0=MULT, op1=ADD)
            else:
                nc.vector.scalar_tensor_tensor(
                    out=t[:].rearrange("p (h w) -> p h w", h=KH), in0=src,
                    scalar=wo[ct][:, tap:tap + 1],
                    in1=t[:].rearrange("p (h w) -> p h w", h=KH), op0=MULT, op1=ADD)
        off.append(t)

    # f16 copies for the LN-stats matmuls (stats precision is not critical)
    off16, sq16 = [], []
    for ct in range(CT):
        t = pool.tile([128, N], F16, tag=f"off16_{ct}")
        nc.scalar.activation(t[:], off[ct][:], AF.Copy)
        off16.append(t)
        s = pool.tile([128, N], F16, tag=f"sq16_{ct}")
        nc.scalar.activation(s[:], off[ct][:], AF.Square)
        sq16.append(s)

    # ---------------- value conv (PE diag-matmuls) + vtab transposes ----------
    val = [pool.tile([128, PIX], F16, tag=f"val_{ct}", name=f"val_{ct}")
           for ct in range(CT)]
    vps_ctx = tc.tile_pool(name="vps", bufs=2, space="PSUM")
    vps = vps_ctx.__enter__()
    vtctx = tc.tile_pool(name="vtp_ps", bufs=2, space="PSUM")
    vtps = vtctx.__enter__()
    vtsctx = tc.tile_pool(name="vtp_sb", bufs=3)
    vtsb = vtsctx.__enter__()
    vtab_writes = []

    def value_ct(ct):
        x3 = xp16[ct][:].rearrange("p (h w) -> p h w", h=HP)
        for chk in range(NVCH):
            r0 = chk * 8  # output row base of this 448-pixel chunk
            ps = vps.tile([128, VCH], F32, tag="vch", space="PSUM")
            for t in range(9):
                dy, dx = t // 3, t % 3
                src = x3[:, r0 + dy:r0 + dy + 8, dx:dx + W]
                nc.tensor.matmul(ps[:].rearrange("p (h w) -> p h w", h=8),
                                 diag_v[ct][t], src,
                                 start=(t == 0), stop=(t == 8),
                                 skip_group_check=True)
            nc.scalar.activation(val[ct][:, chk * VCH:(chk + 1) * VCH], ps[:],
                                 AF.Identity, bias=bv[ct][:, 0:1])

    def vtab_band(ct):
        for g in range(6):
            c0 = g * 512
            tp4 = vtps.tile([128, 512], F16, tag="tp4", space="PSUM")
            for j in range(4):
                nc.tensor.transpose(tp4[:, j * 128:(j + 1) * 128],
                                    val[ct][:, c0 + j * 128:c0 + (j + 1) * 128],
                                    ident[:])
            w4 = vtsb.tile([128, 512], F16, tag="w4")
            nc.scalar.activation(w4[:], tp4[:], AF.Copy)
            out_ap = bass.AP(io["vtab"].tensor, c0 * C + ct * 128,
                             [[C, 128], [C * 128, 4], [1, 128]])
            vtab_writes.append(dma.dma_start(out_ap, w4[:]))
        tp1 = vtps.tile([64, 128], F16, tag="tp1", space="PSUM")
        nc.tensor.transpose(tp1[:], val[ct][:, 3072:3136], ident[:])
        w1 = vtsb.tile([64, 128], F16, tag="w1")
        nc.scalar.activation(w1[:], tp1[:], AF.Copy)
        vtab_writes.append(
            dma.dma_start(io["vtab"][3072:3136, ct * 128:(ct + 1) * 128], w1[:]))

    value_ct(0)
    vtab_band(0)
    value_ct(1)
    vtab_band(1)

    # ---------------- LN stats (PE f16) + pointwise ----------------
    musq = pool.tile([1, N], F32, tag="musq")
    var = pool.tile([1, N], F32, tag="var")
    sd32 = pool.tile([1, N], F32, tag="sd32")
    rstd32 = pool.tile([1, N], F32, tag="rstd32")
    mu16 = pool.tile([1, N], F16, tag="mu16")
    rstd16 = pool.tile([1, N], F16, tag="rstd16")
    mu_b = pool.tile([128, N], F32, tag="mu_b")
    rstd_b = pool.tile([128, N], F32, tag="rstd_b")
    with tc.tile_pool(name="ln_psum", bufs=1, space="PSUM") as lnp:
        st_ps = lnp.tile([1, N], F32, tag="st_ps")
        for sl in (slice(0, 512), slice(512, N)):
            for ct in range(CT):
                nc.tensor.matmul(st_ps[:, sl], ones_rc[:], off16[ct][:, sl],
                                 start=(ct == 0), stop=(ct == CT - 1))
        nc.scalar.activation(musq[:], st_ps[:], AF.Square)
        nc.scalar.activation(mu16[:], st_ps[:], AF.Copy)
        for sl in (slice(0, 512), slice(512, N)):
            for ct in range(CT):
                nc.tensor.matmul(st_ps[:, sl], ones_rc[:], sq16[ct][:, sl],
                                 start=(ct == 0), stop=(ct == CT - 1))
        nc.vector.tensor_tensor(out=var[:], in0=st_ps[:], in1=musq[:], op=SUB)
    nc.scalar.activation(sd32[:], var[:], AF.Sqrt, bias=eps_t[:, 0:1])
    nc.vector.reciprocal_approx_fast(rstd32[:], sd32[:])
    nc.scalar.activation(rstd16[:], rstd32[:], AF.Copy)
    with tc.tile_pool(name="bc_psum", bufs=1, space="PSUM") as bcp:
        bc_ps = bcp.tile([128, N], F32, tag="bc_ps")
        for sl in (slice(0, 512), slice(512, N)):
            nc.tensor.matmul(bc_ps[:, sl], one_row[:], mu16[:, sl],
                             start=True, stop=True)
        nc.scalar.activation(mu_b[:], bc_ps[:], AF.Copy)
        for sl in (slice(0, 512), slice(512, N)):
            nc.tensor.matmul(bc_ps[:, sl], one_row[:], rstd16[:, sl],
                             start=True, stop=True)
        nc.scalar.activation(rstd_b[:], bc_ps[:], AF.Copy)

    # normalize (DVE, fp32) + gelu (scalar, g/b folded into activation)
    gel = []
    for ct in range(CT):
        t1 = off[ct]  # in-place
        nc.vector.tensor_tensor(out=t1[:], in0=t1[:], in1=mu_b[:], op=SUB)
        nc.vector.tensor_tensor(out=t1[:], in0=t1[:], in1=rstd_b[:], op=MULT)
        g = pool.tile([128, N], F32, tag=f"gel_{ct}")
        nc.scalar.activation(g[:], t1[:], AF.Gelu,
                             scale=lng[ct][:, 0:1], bias=lnb[ct][:, 0:1])
        gel.append(g)

    value_ct(2)
    vtab_band(2)
    vtsctx.__exit__(None, None, None)
    vtctx.__exit__(None, None, None)
    vps_ctx.__exit__(None, None, None)

    # ---------------- offset head: w2t matmul (fp32) + tanh + ixy ----------------
    pos = pool.tile([2, N], F32, tag="pos")
    tnh = pool.tile([2, N], F32, tag="tnh")
    ixy0 = pool.tile([2, N], F32, tag="ixy0")
    with tc.tile_pool(name="off_psum", bufs=1, space="PSUM") as offp:
        oyx_ps = offp.tile([2, N], F32, tag="oyx")
        for sl in (slice(0, 512), slice(512, N)):
            for ct in range(CT):
                nc.tensor.matmul(oyx_ps[:, sl], w2t32[ct][:], gel[ct][:, sl],
                                 start=(ct == 0), stop=(ct == CT - 1))
        oyx_sb = pool.tile([2, N], F32, tag="oyx_sb")
        nc.scalar.activation(oyx_sb[:], oyx_ps[:], AF.Copy)
    nc.vector.tensor_tensor(out=pos[:], in0=oyx_sb[:], in1=refyx[:], op=ADD)
    nc.scalar.activation(tnh[:], pos[:], AF.Tanh)
    # iy/ix - 0.5 = tanh*27.5 + 27.0  (the -0.5 shift makes round() act as floor)
    nc.vector.tensor_scalar(out=ixy0[:], in0=tnh[:], scalar1=(H - 1) / 2.0,
                            scalar2=(H - 1) / 2.0 - 0.5, op0=MULT, op1=ADD)
    ixy_write = dma.dma_start(io["ixy_dram"][:, :], ixy0[:])

    # ---------------- index math (DVE, wide tiles) ----------------
    # layout [112 pts, 14]: cols 0..6 = iy-0.5 per k-tile, cols 7..13 = ix-0.5
    iyx = pool.tile([NT, 2 * NTILES], F32, tag="iyx")
    for j in range(2):
        src = bass.AP(io["ixy_dram"].tensor, j * N, [[1, NT], [NT, NTILES]])
        rd = dma.dma_start(iyx[:, j * NTILES:(j + 1) * NTILES], src)
        tile.add_dep_helper(rd.ins, ixy_write.ins, reason="ixy dram RAW")
    x0i = pool.tile([NT, 2 * NTILES], I32, tag="x0i")
    nc.vector.tensor_copy(x0i[:], iyx[:])   # round(v-0.5) == floor(v)
    x0f = pool.tile([NT, 2 * NTILES], F32, tag="x0f")
    nc.vector.tensor_copy(x0f[:], x0i[:])
    nc.vector.tensor_scalar(out=x0f[:], in0=x0f[:], scalar1=float(H - 2),
                            scalar2=0.0, op0=mybir.AluOpType.min,
                            op1=mybir.AluOpType.max)
    ys, xs = slice(0, NTILES), slice(NTILES, 2 * NTILES)
    idxf = pool.tile([NT, NTILES], F32, tag="idxf")
    nc.vector.tensor_scalar(out=idxf[:], in0=x0f[:, ys], scalar1=float(W),
                            scalar2=None, op0=MULT)
    nc.vector.tensor_tensor(out=idxf[:], in0=idxf[:], in1=x0f[:, xs], op=ADD)
    idxi = pool.tile([NT, NTILES], I32, tag="idxi")
    nc.vector.tensor_copy(idxi[:], idxf[:])
    frac = pool.tile([NT, 2 * NTILES], F32, tag="frac")
    nc.vector.tensor_tensor(out=frac[:], in0=iyx[:], in1=x0f[:], op=SUB)
    nc.vector.tensor_scalar_add(frac[:], frac[:], 0.5)
    omf = pool.tile([NT, 2 * NTILES], F32, tag="omf")
    nc.vector.tensor_scalar(out=omf[:], in0=frac[:], scalar1=-1.0, scalar2=1.0,
                            op0=MULT, op1=ADD)
    wts = [pool.tile([NT, NTILES], F32, tag=f"wts{j}", name=f"wts{j}") for j in range(4)]
    nc.vector.tensor_tensor(out=wts[0][:], in0=omf[:, ys], in1=omf[:, xs], op=MULT)
    nc.vector.tensor_tensor(out=wts[1][:], in0=omf[:, ys], in1=frac[:, xs], op=MULT)
    nc.vector.tensor_tensor(out=wts[2][:], in0=frac[:, ys], in1=omf[:, xs], op=MULT)
    nc.vector.tensor_tensor(out=wts[3][:], in0=frac[:, ys], in1=frac[:, xs], op=MULT)

    # ---------------- query conv (PE diag-matmuls, 392-pixel chunks) ----------
    diag_q = make_diag(wq, "q")
    q16 = [pool.tile([128, HALF_PIX], F16, tag=f"q_{ct}", name=f"q_{ct}")
           for ct in range(CT)]
    with tc.tile_pool(name="qps", bufs=2, space="PSUM") as qps:
        for ct in range(CT):
            x3 = xq16[ct][:].rearrange("p (h w) -> p h w", h=30)
            for chk in range(4):
                r0 = chk * 7
                ps = qps.tile([128, 392], F32, tag="qch", space="PSUM")
                for t in range(9):
                    dy, dx = t // 3, t % 3
                    src = x3[:, r0 + dy:r0 + dy + 7, dx:dx + W]
                    nc.tensor.matmul(ps[:].rearrange("p (h w) -> p h w", h=7),
                                     diag_q[ct][t], src,
                                     start=(t == 0), stop=(t == 8),
                                     skip_group_check=True)
                nc.scalar.activation(q16[ct][:, chk * 392:(chk + 1) * 392], ps[:],
                                     AF.Identity, bias=bq[ct][:, 0:1])

    # ---------------- key conv (PE diag-matmuls, stride-2 windows) ------------
    diag_k = make_diag(wk, "k")
    key = [pool.tile([128, N], F16, tag=f"key_{ct}", name=f"key_{ct}")
           for ct in range(CT)]
    with tc.tile_pool(name="kps", bufs=2, space="PSUM") as kps:
        for ct in range(CT):
            x3 = xp16[ct][:].rearrange("p (h w) -> p h w", h=HP)
            for chk in range(2):
                r0 = chk * 14
                ps = kps.tile([128, 392], F32, tag="kch", space="PSUM")
                for t in range(9):
                    dy, dx = t // 3, t % 3
                    src = x3[:, 2 * r0 + dy:2 * r0 + dy + 27:2,
                             dx:dx + 2 * KW - 1:2]
                    nc.tensor.matmul(ps[:].rearrange("p (h w) -> p h w", h=14),
                                     diag_k[ct][t], src,
                                     start=(t == 0), stop=(t == 8),
                                     skip_group_check=True)
                nc.scalar.activation(key[ct][:, chk * 392:(chk + 1) * 392], ps[:],
                                     AF.Identity, bias=bk[ct][:, 0:1])

    # kT (PE transpose)
    kT = []
    with tc.tile_pool(name="ktp", bufs=3, space="PSUM") as ktp:
        for k in range(NTILES):
            t = pool.tile([NT, C], F16, tag=f"kT_{k}")
            for ct in range(CT):
                ps = ktp.tile([NT, 128], F16, tag="kt_ps", space="PSUM")
                nc.tensor.transpose(ps[:], key[ct][:, k * NT:(k + 1) * NT], ident[:])
                nc.scalar.activation(t[:, ct * 128:(ct + 1) * 128], ps[:], AF.Copy)
            kT.append(t)

    # ---------------- gathers + bilinear + M (PSUM-accumulated) ----------------
    vs = []
    with tc.tile_pool(name="m_psum", bufs=1, space="PSUM") as mps, \
         tc.tile_pool(name="gat", bufs=3) as gat:
        m_ps = [mps.tile([128, HC], F32, tag=f"m_ps{i}", name=f"m_ps{i}")
                for i in range(CT)]
        for k in range(NTILES):
            g0 = gat.tile([NT, 2 * C], F16, tag="g0")
            g1 = gat.tile([NT, 2 * C], F16, tag="g1")
            for g, delta in ((g0, 0), (g1, W)):
                gi = gp.indirect_dma_start(
                    out=g[:], out_offset=None, in_=io["vtab"][:, :],
                    in_offset=bass.IndirectOffsetOnAxis(ap=idxi[:, k:k + 1], axis=0),
                    element_offset=delta * C,
                    bounds_check=PIX - 1, oob_is_err=False)
                for wi in vtab_writes:
                    tile.add_dep_helper(gi.ins, wi.ins, reason="vtab RAW")
            v = pool.tile([NT, C], F16, tag=f"vs_{k}")
            nc.vector.tensor_scalar(out=v[:], in0=g0[:, 0:C], scalar1=wts[0][:, k:k + 1],
                                    scalar2=None, op0=MULT)
            nc.vector.scalar_tensor_tensor(out=v[:], in0=g0[:, C:2 * C],
                                           scalar=wts[1][:, k:k + 1], in1=v[:],
                                           op0=MULT, op1=ADD)
            nc.vector.scalar_tensor_tensor(out=v[:], in0=g1[:, 0:C],
                                           scalar=wts[2][:, k:k + 1], in1=v[:],
                                           op0=MULT, op1=ADD)
            nc.vector.scalar_tensor_tensor(out=v[:], in0=g1[:, C:2 * C],
                                           scalar=wts[3][:, k:k + 1], in1=v[:],
                                           op0=MULT, op1=ADD)
            vs.append(v)
            for h in range(NH):
                ct, j = h // 4, h % 4
                nc.tensor.matmul(m_ps[ct][j * 32:(j + 1) * 32, :],
                                 v[:, h * HC:(h + 1) * HC],
                                 kT[k][:, h * HC:(h + 1) * HC],
                                 start=(k == 0), stop=(k == NTILES - 1),
                                 tile_position=(0, j * 32),
                                 skip_group_check=True)
        m16 = []
        for ct in range(CT):
            t = pool.tile([128, HC], F16, tag=f"m16_{ct}")
            nc.scalar.activation(t[:], m_ps[ct][:], AF.Copy, scale=SCALE)
            m16.append(t)

    # ---------------- A^T = blockdiag(scale*M)^T w_out^T, then y ----------------
    at16 = []
    with tc.tile_pool(name="atps", bufs=1, space="PSUM") as atps:
        at_ps = [atps.tile([128, C], F32, tag=f"at_ps{i}", name=f"at_ps{i}")
                 for i in range(CT)]
        for h in range(NH):
            ct, j = h // 4, h % 4
            nc.tensor.matmul(at_ps[ct][j * 32:(j + 1) * 32, :],
                             m16[ct][j * 32:(j + 1) * 32, :],
                             wot16[ct][j * 32:(j + 1) * 32, :],
                             start=True, stop=True,
                             tile_position=(j * 32, j * 32))
        for ct in range(CT):
            t = pool.tile([128, C], F16, tag=f"at16_{ct}")
            nc.scalar.activation(t[:], at_ps[ct][:], AF.Copy)
            at16.append(t)

    NCHUNK = 4
    CW = HALF_PIX // NCHUNK  # 392
    with tc.tile_pool(name="yps", bufs=2, space="PSUM") as yps, \
         tc.tile_pool(name="ysb", bufs=3) as ysb:
        for ot in range(CT):
            for ch in range(NCHUNK):
                y_ps = yps.tile([128, CW], F32, tag="y_ps", space="PSUM")
                for ct in range(CT):
                    nc.tensor.matmul(y_ps[:], at16[ct][:, ot * 128:(ot + 1) * 128],
                                     q16[ct][:, ch * CW:(ch + 1) * CW],
                                     start=(ct == 0), stop=(ct == CT - 1))
                y_sb = ysb.tile([128, CW], F32, tag="y_sb")
                nc.scalar.activation(y_sb[:], y_ps[:], AF.Copy)
                dma.dma_start(io["y"][ot * 128:(ot + 1) * 128, ch * CW:(ch + 1) * CW],
                              y_sb[:])


def build_program():
    if "nc" in _CACHE:
        return _CACHE["nc"]
    nc = bacc.Bacc("TRN2", target_bir_lowering=False, debug=False, num_devices=8)
    io = {}
    io["xp16"] = nc.dram_tensor("xp16", (C, HP * HP), F16, kind="ExternalInput").ap()
    io["xq"] = nc.dram_tensor("xq", (C, 30 * HP), F16, kind="ExternalInput").ap()
    for nm, shape in [("wv", (C, 9)), ("wq", (C, 9)), ("wk", (C, 9)), ("wo", (C, 9)),
                      ("bv", (C, 1)), ("bq", (C, 1)), ("bk", (C, 1)), ("bo", (C, 1)),
                      ("lng", (C, 1)), ("lnb", (C, 1)), ("w2t", (C, 2)),
                      ("refyx", (2, N))]:
        io[nm] = nc.dram_tensor(nm, shape, F32, kind="ExternalInput").ap()
    io["wot"] = nc.dram_tensor("wot", (C, C), F16, kind="ExternalInput").ap()
    io["vtab"] = nc.dram_tensor("vtab", (PIX, C), F16).ap()
    io["ixy_dram"] = nc.dram_tensor("ixy_dram", (2, N), F32).ap()
    io["y"] = nc.dram_tensor("y", (C, HALF_PIX), F32, kind="ExternalOutput").ap()

    with tile.TileContext(nc) as tc:
        with contextlib.ExitStack() as ctx:
            _emit(nc, tc, ctx, io)
    nc.compile()
    _CACHE["nc"] = nc
    return nc


def host_prep(inputs):
    """Build the 8 per-core input maps from full inputs."""
    x = np.asarray(inputs["x"], np.float32)          # (B, C, H, W)
    xpad = np.pad(x, ((0, 0), (0, 0), (1, 1), (1, 1)))  # (B, C, 58, 58)
    shared = {}
    for nm, src in [("wv", "w_v"), ("wq", "w_q"), ("wk", "w_k"), ("wo", "w_off1")]:
        shared[nm] = np.asarray(inputs[src], np.float32).reshape(C, 9)
    for nm, src in [("bv", "b_v"), ("bq", "b_q"), ("bk", "b_k"), ("bo", "b_off1"),
                    ("lng", "ln_g"), ("lnb", "ln_b")]:
        shared[nm] = np.asarray(inputs[src], np.float32).reshape(C, 1)
    shared["w2t"] = np.ascontiguousarray(np.asarray(inputs["w_off2"], np.float32).T)
    shared["wot"] = np.ascontiguousarray(
        np.asarray(inputs["w_out"], np.float32).T).astype(np.float16)   # (C,C) [c,o]
    ry = (np.arange(KH, dtype=np.float32) + 0.5) / KH * 2 - 1
    rx = (np.arange(KW, dtype=np.float32) + 0.5) / KW * 2 - 1
    refyx = np.stack([np.repeat(ry, KW), np.tile(rx, KH)])   # (2, 784), row0=y
    shared["refyx"] = np.ascontiguousarray(refyx, dtype=np.float32)

    in_maps = []
    for core in range(8):
        b, half = core // 2, core % 2
        m = dict(shared)
        xb = xpad[b]
        m["xp16"] = np.ascontiguousarray(xb.reshape(C, HP * HP)).astype(np.float16)
        r0 = half * HALF_ROWS
        m["xq"] = np.ascontiguousarray(
            xb[:, r0:r0 + 30, :].reshape(C, 30 * HP)).astype(np.float16)
        in_maps.append(m)
    return in_maps


def assemble(results):
    y = np.empty((B, C, H, W), np.float32)
    for core in range(8):
        b, half = core // 2, core % 2
        part = results[core]["y"].reshape(C, HALF_ROWS, W)
        y[b, :, half * HALF_ROWS:(half + 1) * HALF_ROWS, :] = part
    return y


def run(inputs, trace=False):
    nc = build_program()
    in_maps = host_prep(inputs)
    res = run_bass_kernel_spmd(nc, in_maps, core_ids=list(range(8)), trace=trace)
    return assemble(res.results), res


def kernel(**inputs):
    out, _ = run(inputs, trace=False)
    return out


# revision 22
# speedup vs baseline: 2.5786x; 1.2011x over previous
"""Trainium2 Bass kernel for nn_DeformableAttention (B=4, C=384, H=W=56, NH=12, HC=32, STRIDE=2).

Self-contained: hardcodes shapes/sharding. Sharding: 8 cores = 4 batches x 2
pixel-row-halves. Each core computes the full value/key/offset branches for its
batch (duplicated across the pair) and the query branch + final GEMM for its
half of the 3136 output pixels.

Math note: the reference computes out = (scale * q^T k) v^T without softmax, so
attention is linear and reassociates:
    y[b] = (w_out @ blockdiag_h(scale * M[b,h])) @ Q[b],
    M[b,h] = V_s[b,h] K[b,h]^T  (32x32 per head)
which drops the 48x(3136x784x32) einsums to a few small GEMMs.

Measured HW model (trace-derived): DVE ~1.1ns/elem regardless of dtype/stride;
PE matmul ~M+128 cycles; Pool elementwise ~2ns/elem with a one-time ~60us
ucode-load on the first op (pre-warmed with dummies); per-DMA-queue bandwidth
~90GB/s (big loads are split into slices to ride multiple queues).

Schedule: value conv runs on the PE as 9 diag(w_tap) matmuls PSUM-accumulated
per 448-pixel chunk (bias folded into the PSUM->SBUF copy), freeing ~92us of
DVE. The off branch is fp32 end-to-end (sample positions are precision
critical). DVE keeps off/key/query convs + LN pointwise + bilinear. Gathers
fetch (x0,x0+1) pixel pairs as one 768-elem row. M accumulates over the 7
k-tiles directly in PSUM. floor() is computed via round(x-0.5) (casts round to
nearest; integer ties land on the complementary-weight corner, which is exact).
"""
import contextlib

import numpy as np

import concourse.bass as bass
import concourse.tile as tile
from concourse import bacc, mybir
from concourse.bass_utils import run_bass_kernel_spmd
from concourse.masks import make_identity

F32, F16, I32 = mybir.dt.float32, mybir.dt.float16, mybir.dt.int32
MULT, ADD, SUB = mybir.AluOpType.mult, mybir.AluOpType.add, mybir.AluOpType.subtract
AF = mybir.ActivationFunctionType

B, C, H, W = 4, 384, 56, 56
NH, HC = 12, 32
SCALE = HC ** -0.5
HP = H + 2                      # 58 padded
PIX = H * W                     # 3136
KH = KW = 28                    # stride-2 output
N = KH * KW                     # 784
NT = 112                        # point-tile size (7 tiles)
NTILES = N // NT
HALF_ROWS = H // 2              # 28
HALF_PIX = HALF_ROWS * W        # 1568
CT = C // 128                   # 3 channel tiles
EPS = 1e-5
VCH = 448                       # value-conv PE chunk (8 rows of 56)
NVCH = PIX // VCH               # 7 chunks per ct

_CACHE = {}


def _emit(nc, tc, ctx, io):
    pool = ctx.enter_context(tc.tile_pool(name="main", bufs=1))
    dma = nc.sync
    gp = nc.gpsimd

    # ---------------- loads (big tensors split across DMA queues) ----------------
    def load_split(name, width, dtype, nsplit, eng):
        out = []
        for ct in range(CT):
            t = pool.tile([128, width], dtype, tag=f"{name}_{ct}")
            step = 128 // nsplit
            for s in range(nsplit):
                r0 = s * step
                eng.dma_start(t[r0:r0 + step, :],
                              io[name][ct * 128 + r0:ct * 128 + r0 + step, :])
            out.append(t)
        return out

    def load_cols(name, width, dtype=F32, eng=dma):
        out = []
        for ct in range(CT):
            t = pool.tile([128, width], dtype, tag=f"{name}_{ct}")
            eng.dma_start(t[:], io[name][ct * 128:(ct + 1) * 128, :])
            out.append(t)
        return out

    xp16 = load_split("xp16", HP * HP, F16, 4, dma)
    wv = load_cols("wv", 9)
    bv = load_cols("bv", 1)
    wo = load_cols("wo", 9)
    bo = load_cols("bo", 1)
    wk = load_cols("wk", 9)
    bk = load_cols("bk", 1)
    xq16 = load_split("xq", 30 * HP, F16, 2, dma)
    wq = load_cols("wq", 9)
    bq = load_cols("bq", 1)
    lng = load_cols("lng", 1)
    lnb = load_cols("lnb", 1)
    w2t32 = load_cols("w2t", 2)
    wot16 = load_cols("wot", C, dtype=F16, eng=dma)
    refyx = pool.tile([2, N], F32, tag="refyx")
    dma.dma_start(refyx[:], io["refyx"][:, :])
    ones_rc = pool.tile([128, 1], F16, tag="ones_rc")
    nc.vector.memset(ones_rc[:], 1.0 / C)
    one_row = pool.tile([1, 128], F16, tag="one_row")
    nc.vector.memset(one_row[:], 1.0)
    ident = pool.tile([128, 128], F16, tag="ident")
    make_identity(nc, ident[:])
    eps_t = pool.tile([1, 1], F32, tag="eps_t")
    nc.vector.memset(eps_t[:], EPS)

    # diag(w_tap) tiles (scalar engine: per-partition scale of the identity)
    def make_diag(w, nm):
        out = []
        for ct in range(CT):
            dd = []
            for t in range(9):
                d = pool.tile([128, 128], F16, tag=f"dg_{nm}_{ct}_{t}",
                              name=f"dg_{nm}_{ct}_{t}")
                nc.scalar.activation(d[:], ident[:], AF.Copy, scale=w[ct][:, t:t + 1])
                dd.append(d)
            out.append(dd)
        return out

    diag_v = make_diag(wv, "v")

    # ---------------- off conv (DVE, fp32 accumulate) ----------------
    off = []
    for ct in range(CT):
        t = pool.tile([128, N], F32, tag=f"off_{ct}")
        x3 = xp16[ct][:].rearrange("p (h w) -> p h w", h=HP)
        for tap in range(9):
            dy, dx = tap // 3, tap % 3
            src = x3[:, dy:dy + 2 * KH - 1:2, dx:dx + 2 * KW - 1:2]
            if tap == 0:
                nc.vector.tensor_scalar(out=t[:].rearrange("p (h w) -> p h w", h=KH),
                                        in0=src, scalar1=wo[ct][:, 0:1],
                                        scalar2=bo[ct][:, 0:1], op0=MULT, op1=ADD)
            else:
                nc.vector.scalar_tensor_tensor(
                    out=t[:].rearrange("p (h w) -> p h w", h=KH), in0=src,
                    scalar=wo[ct][:, tap:tap + 1],
                    in1=t[:].rearrange("p (h w) -> p h w", h=KH), op0=MULT, op1=ADD)
        off.append(t)

    # f16 copies for the LN-stats matmuls (stats precision is not critical)
    off16, sq16 = [], []
    for ct in range(CT):
        t = pool.tile([128, N], F16, tag=f"off16_{ct}")
        nc.scalar.activation(t[:], off[ct][:], AF.Copy)
        off16.append(t)
        s = pool.tile([128, N], F16, tag=f"sq16_{ct}")
        nc.scalar.activation(s[:], off[ct][:], AF.Square)
        sq16.append(s)

    # ---------------- value conv (PE diag-matmuls) + vtab transposes ----------
    val = [pool.tile([128, PIX], F16, tag=f"val_{ct}", name=f"val_{ct}")
           for ct in range(CT)]
    vps_ctx = tc.tile_pool(name="vps", bufs=2, space="PSUM")
    vps = vps_ctx.__enter__()
    vtctx = tc.tile_pool(name="vtp_ps", bufs=2, space="PSUM")
    vtps = vtctx.__enter__()
    vtsctx = tc.tile_pool(name="vtp_sb", bufs=3)
    vtsb = vtsctx.__enter__()
    vtab_writes = []

    def value_ct(ct):
        x3 = xp16[ct][:].rearrange("p (h w) -> p h w", h=HP)
        for chk in range(NVCH):
            r0 = chk * 8  # output row base of this 448-pixel chunk
            ps = vps.tile([128, VCH], F32, tag="vch", space="PSUM")
            for t in range(9):
                dy, dx = t // 3, t % 3
                src = x3[:, r0 + dy:r0 + dy + 8, dx:dx + W]
                nc.tensor.matmul(ps[:].rearrange("p (h w) -> p h w", h=8),
                                 diag_v[ct][t], src,
                                 start=(t == 0), stop=(t == 8),
                                 skip_group_check=True)
            nc.scalar.activation(val[ct][:, chk * VCH:(chk + 1) * VCH], ps[:],
                                 AF.Identity, bias=bv[ct][:, 0:1])

    def vtab_band(ct):
        for g in range(6):
            c0 = g * 512
            tp4 = vtps.tile([128, 512], F16, tag="tp4", space="PSUM")
            for j in range(4):
                nc.tensor.transpose(tp4[:, j * 128:(j + 1) * 128],
                                    val[ct][:, c0 + j * 128:c0 + (j + 1) * 128],
                                    ident[:])
            w4 = vtsb.tile([128, 512], F16, tag="w4")
            nc.scalar.activation(w4[:], tp4[:], AF.Copy)
            out_ap = bass.AP(io["vtab"].tensor, c0 * C + ct * 128,
                             [[C, 128], [C * 128, 4], [1, 128]])
            vtab_writes.append(dma.dma_start(out_ap, w4[:]))
        tp1 = vtps.tile([64, 128], F16, tag="tp1", space="PSUM")
        nc.tensor.transpose(tp1[:], val[ct][:, 3072:3136], ident[:])
        w1 = vtsb.tile([64, 128], F16, tag="w1")
        nc.scalar.activation(w1[:], tp1[:], AF.Copy)
        vtab_writes.append(
            dma.dma_start(io["vtab"][3072:3136, ct * 128:(ct + 1) * 128], w1[:]))

    value_ct(0)
    vtab_band(0)
    value_ct(1)
    vtab_band(1)

    # ---------------- LN stats (PE f16) + pointwise ----------------
    musq = pool.tile([1, N], F32, tag="musq")
    var = pool.tile([1, N], F32, tag="var")
    sd32 = pool.tile([1, N], F32, tag="sd32")
    rstd32 = pool.tile([1, N], F32, tag="rstd32")
    mu16 = pool.tile([1, N], F16, tag="mu16")
    rstd16 = pool.tile([1, N], F16, tag="rstd16")
    mu_b = pool.tile([128, N], F32, tag="mu_b")
    rstd_b = pool.tile([128, N], F32, tag="rstd_b")
    with tc.tile_pool(name="ln_psum", bufs=1, space="PSUM") as lnp:
        st_ps = lnp.tile([1, N], F32, tag="st_ps")
        for sl in (slice(0, 512), slice(512, N)):
            for ct in range(CT):
                nc.tensor.matmul(st_ps[:, sl], ones_rc[:], off16[ct][:, sl],
                                 start=(ct == 0), stop=(ct == CT - 1))
        nc.scalar.activation(musq[:], st_ps[:], AF.Square)
        nc.scalar.activation(mu16[:], st_ps[:], AF.Copy)
        for sl in (slice(0, 512), slice(512, N)):
            for ct in range(CT):
                nc.tensor.matmul(st_ps[:, sl], ones_rc[:], sq16[ct][:, sl],
                                 start=(ct == 0), stop=(ct == CT - 1))
        nc.vector.tensor_tensor(out=var[:], in0=st_ps[:], in1=musq[:], op=SUB)
    nc.scalar.activation(sd32[:], var[:], AF.Sqrt, bias=eps_t[:, 0:1])
    nc.vector.reciprocal_approx_fast(rstd32[:], sd32[:])
    nc.scalar.activation(rstd16[:], rstd32[:], AF.Copy)
    with tc.tile_pool(name="bc_psum", bufs=1, space="PSUM") as bcp:
        bc_ps = bcp.tile([128, N], F32, tag="bc_ps")
        for sl in (slice(0, 512), slice(512, N)):
            nc.tensor.matmul(bc_ps[:, sl], one_row[:], mu16[:, sl],
                             start=True, stop=True)
        nc.scalar.activation(mu_b[:], bc_ps[:], AF.Copy)
        for sl in (slice(0, 512), slice(512, N)):
            nc.tensor.matmul(bc_ps[:, sl], one_row[:], rstd16[:, sl],
                             start=True, stop=True)
        nc.scalar.activation(rstd_b[:], bc_ps[:], AF.Copy)

    # normalize (DVE, fp32) + gelu (scalar, g/b folded into activation)
    gel = []
    for ct in range(CT):
        t1 = off[ct]  # in-place
        nc.vector.tensor_tensor(out=t1[:], in0=t1[:], in1=mu_b[:], op=SUB)
        nc.vector.tensor_tensor(out=t1[:], in0=t1[:], in1=rstd_b[:], op=MULT)
        g = pool.tile([128, N], F32, tag=f"gel_{ct}")
        nc.scalar.activation(g[:], t1[:], AF.Gelu,
                             scale=lng[ct][:, 0:1], bias=lnb[ct][:, 0:1])
        gel.append(g)

    # ---------------- key conv (DVE, strided taps) ----------------
    key = []
    for ct in range(CT):
        t = pool.tile([128, N], F16, tag=f"key_{ct}")
        x3 = xp16[ct][:].rearrange("p (h w) -> p h w", h=HP)
        for tap in range(9):
            dy, dx = tap // 3, tap % 3
            src2 = x3[:, dy:dy + 2 * KH - 1:2, dx:dx + 2 * KW - 1:2]
            o2 = t[:].rearrange("p (h w) -> p h w", h=KH)
            if tap == 0:
                nc.vector.tensor_scalar(out=o2, in0=src2, scalar1=wk[ct][:, 0:1],
                                        scalar2=bk[ct][:, 0:1], op0=MULT, op1=ADD)
            else:
                nc.vector.scalar_tensor_tensor(out=o2, in0=src2,
                                               scalar=wk[ct][:, tap:tap + 1],
                                               in1=o2, op0=MULT, op1=ADD)
        key.append(t)


    value_ct(2)
    vtab_band(2)
    vtsctx.__exit__(None, None, None)
    vtctx.__exit__(None, None, None)
    vps_ctx.__exit__(None, None, None)

    # ---------------- offset head: w2t matmul (fp32) + tanh + ixy ----------------
    pos = pool.tile([2, N], F32, tag="pos")
    tnh = pool.tile([2, N], F32, tag="tnh")
    ixy0 = pool.tile([2, N], F32, tag="ixy0")
    with tc.tile_pool(name="off_psum", bufs=1, space="PSUM") as offp:
        oyx_ps = offp.tile([2, N], F32, tag="oyx")
        for sl in (slice(0, 512), slice(512, N)):
            for ct in range(CT):
                nc.tensor.matmul(oyx_ps[:, sl], w2t32[ct][:], gel[ct][:, sl],
                                 start=(ct == 0), stop=(ct == CT - 1))
        oyx_sb = pool.tile([2, N], F32, tag="oyx_sb")
        nc.scalar.activation(oyx_sb[:], oyx_ps[:], AF.Copy)
    nc.vector.tensor_tensor(out=pos[:], in0=oyx_sb[:], in1=refyx[:], op=ADD)
    nc.scalar.activation(tnh[:], pos[:], AF.Tanh)
    # iy/ix - 0.5 = tanh*27.5 + 27.0  (the -0.5 shift makes round() act as floor)
    nc.vector.tensor_scalar(out=ixy0[:], in0=tnh[:], scalar1=(H - 1) / 2.0,
                            scalar2=(H - 1) / 2.0 - 0.5, op0=MULT, op1=ADD)
    ixy_write = dma.dma_start(io["ixy_dram"][:, :], ixy0[:])

    # ---------------- index math (DVE, wide tiles) ----------------
    # layout [112 pts, 14]: cols 0..6 = iy-0.5 per k-tile, cols 7..13 = ix-0.5
    iyx = pool.tile([NT, 2 * NTILES], F32, tag="iyx")
    for j in range(2):
        src = bass.AP(io["ixy_dram"].tensor, j * N, [[1, NT], [NT, NTILES]])
        rd = dma.dma_start(iyx[:, j * NTILES:(j + 1) * NTILES], src)
        tile.add_dep_helper(rd.ins, ixy_write.ins, reason="ixy dram RAW")
    x0i = pool.tile([NT, 2 * NTILES], I32, tag="x0i")
    nc.vector.tensor_copy(x0i[:], iyx[:])   # round(v-0.5) == floor(v)
    x0f = pool.tile([NT, 2 * NTILES], F32, tag="x0f")
    nc.vector.tensor_copy(x0f[:], x0i[:])
    nc.vector.tensor_scalar(out=x0f[:], in0=x0f[:], scalar1=float(H - 2),
                            scalar2=0.0, op0=mybir.AluOpType.min,
                            op1=mybir.AluOpType.max)
    ys, xs = slice(0, NTILES), slice(NTILES, 2 * NTILES)
    idxf = pool.tile([NT, NTILES], F32, tag="idxf")
    nc.vector.tensor_scalar(out=idxf[:], in0=x0f[:, ys], scalar1=float(W),
                            scalar2=None, op0=MULT)
    nc.vector.tensor_tensor(out=idxf[:], in0=idxf[:], in1=x0f[:, xs], op=ADD)
    idxi = pool.tile([NT, NTILES], I32, tag="idxi")
    nc.vector.tensor_copy(idxi[:], idxf[:])
    frac = pool.tile([NT, 2 * NTILES], F32, tag="frac")
    nc.vector.tensor_tensor(out=frac[:], in0=iyx[:], in1=x0f[:], op=SUB)
    nc.vector.tensor_scalar_add(frac[:], frac[:], 0.5)
    omf = pool.tile([NT, 2 * NTILES], F32, tag="omf")
    nc.vector.tensor_scalar(out=omf[:], in0=frac[:], scalar1=-1.0, scalar2=1.0,
                            op0=MULT, op1=ADD)
    wts = [pool.tile([NT, NTILES], F32, tag=f"wts{j}", name=f"wts{j}") for j in range(4)]
    nc.vector.tensor_tensor(out=wts[0][:], in0=omf[:, ys], in1=omf[:, xs], op=MULT)
    nc.vector.tensor_tensor(out=wts[1][:], in0=omf[:, ys], in1=frac[:, xs], op=MULT)
    nc.vector.tensor_tensor(out=wts[2][:], in0=frac[:, ys], in1=omf[:, xs], op=MULT)
    nc.vector.tensor_tensor(out=wts[3][:], in0=frac[:, ys], in1=frac[:, xs], op=MULT)

    # ---------------- query conv (PE diag-matmuls, 392-pixel chunks) ----------
    diag_q = make_diag(wq, "q")
    q16 = [pool.tile([128, HALF_PIX], F16, tag=f"q_{ct}", name=f"q_{ct}")
           for ct in range(CT)]
    with tc.tile_pool(name="qps", bufs=2, space="PSUM") as qps:
        for ct in range(CT):
            x3 = xq16[ct][:].rearrange("p (h w) -> p h w", h=30)
            for chk in range(4):
                r0 = chk * 7
                ps = qps.tile([128, 392], F32, tag="qch", space="PSUM")
                for t in range(9):
                    dy, dx = t // 3, t % 3
                    src = x3[:, r0 + dy:r0 + dy + 7, dx:dx + W]
                    nc.tensor.matmul(ps[:].rearrange("p (h w) -> p h w", h=7),
                                     diag_q[ct][t], src,
                                     start=(t == 0), stop=(t == 8),
                                     skip_group_check=True)
                nc.scalar.activation(q16[ct][:, chk * 392:(chk + 1) * 392], ps[:],
                                     AF.Identity, bias=bq[ct][:, 0:1])

    # kT (PE transpose)
    kT = []
    with tc.tile_pool(name="ktp", bufs=3, space="PSUM") as ktp:
        for k in range(NTILES):
            t = pool.tile([NT, C], F16, tag=f"kT_{k}")
            for ct in range(CT):
                ps = ktp.tile([NT, 128], F16, tag="kt_ps", space="PSUM")
                nc.tensor.transpose(ps[:], key[ct][:, k * NT:(k + 1) * NT], ident[:])
                nc.scalar.activation(t[:, ct * 128:(ct + 1) * 128], ps[:], AF.Copy)
            kT.append(t)

    # ---------------- gathers + bilinear + M (PSUM-accumulated) ----------------
    vs = []
    with tc.tile_pool(name="m_psum", bufs=1, space="PSUM") as mps, \
         tc.tile_pool(name="gat", bufs=3) as gat:
        m_ps = [mps.tile([128, HC], F32, tag=f"m_ps{i}", name=f"m_ps{i}")
                for i in range(CT)]
        for k in range(NTILES):
            g0 = gat.tile([NT, 2 * C], F16, tag="g0")
            g1 = gat.tile([NT, 2 * C], F16, tag="g1")
            for g, delta in ((g0, 0), (g1, W)):
                gi = gp.indirect_dma_start(
                    out=g[:], out_offset=None, in_=io["vtab"][:, :],
                    in_offset=bass.IndirectOffsetOnAxis(ap=idxi[:, k:k + 1], axis=0),
                    element_offset=delta * C,
                    bounds_check=PIX - 1, oob_is_err=False)
                for wi in vtab_writes:
                    tile.add_dep_helper(gi.ins, wi.ins, reason="vtab RAW")
            v = pool.tile([NT, C], F16, tag=f"vs_{k}")
            nc.vector.tensor_scalar(out=v[:], in0=g0[:, 0:C], scalar1=wts[0][:, k:k + 1],
                                    scalar2=None, op0=MULT)
            nc.vector.scalar_tensor_tensor(out=v[:], in0=g0[:, C:2 * C],
                                           scalar=wts[1][:, k:k + 1], in1=v[:],
                                           op0=MULT, op1=ADD)
            nc.vector.scalar_tensor_tensor(out=v[:], in0=g1[:, 0:C],
                                           scalar=wts[2][:, k:k + 1], in1=v[:],
                                           op0=MULT, op1=ADD)
            nc.vector.scalar_tensor_tensor(out=v[:], in0=g1[:, C:2 * C],
                                           scalar=wts[3][:, k:k + 1], in1=v[:],
                                           op0=MULT, op1=ADD)
            vs.append(v)
            for h in range(NH):
                ct, j = h // 4, h % 4
                nc.tensor.matmul(m_ps[ct][j * 32:(j + 1) * 32, :],
                                 v[:, h * HC:(h + 1) * HC],
                                 kT[k][:, h * HC:(h + 1) * HC],
                                 start=(k == 0), stop=(k == NTILES - 1),
                                 tile_position=(0, j * 32),
                                 skip_group_check=True)
        m16 = []
        for ct in range(CT):
            t = pool.tile([128, HC], F16, tag=f"m16_{ct}")
            nc.scalar.activation(t[:], m_ps[ct][:], AF.Copy, scale=SCALE)
            m16.append(t)

    # ---------------- A^T = blockdiag(scale*M)^T w_out^T, then y ----------------
    at16 = []
    with tc.tile_pool(name="atps", bufs=1, space="PSUM") as atps:
        at_ps = [atps.tile([128, C], F32, tag=f"at_ps{i}", name=f"at_ps{i}")
                 for i in range(CT)]
        for h in range(NH):
            ct, j = h // 4, h % 4
            nc.tensor.matmul(at_ps[ct][j * 32:(j + 1) * 32, :],
                             m16[ct][j * 32:(j + 1) * 32, :],
                             wot16[ct][j * 32:(j + 1) * 32, :],
                             start=True, stop=True,
                             tile_position=(j * 32, j * 32))
        for ct in range(CT):
            t = pool.tile([128, C], F16, tag=f"at16_{ct}")
            nc.scalar.activation(t[:], at_ps[ct][:], AF.Copy)
            at16.append(t)

    NCHUNK = 4
    CW = HALF_PIX // NCHUNK  # 392
    with tc.tile_pool(name="yps", bufs=2, space="PSUM") as yps, \
         tc.tile_pool(name="ysb", bufs=3) as ysb:
        for ot in range(CT):
            for ch in range(NCHUNK):
                y_ps = yps.tile([128, CW], F32, tag="y_ps", space="PSUM")
                for ct in range(CT):
                    nc.tensor.matmul(y_ps[:], at16[ct][:, ot * 128:(ot + 1) * 128],
                                     q16[ct][:, ch * CW:(ch + 1) * CW],
                                     start=(ct == 0), stop=(ct == CT - 1))
                y_sb = ysb.tile([128, CW], F32, tag="y_sb")
                nc.scalar.activation(y_sb[:], y_ps[:], AF.Copy)
                dma.dma_start(io["y"][ot * 128:(ot + 1) * 128, ch * CW:(ch + 1) * CW],
                              y_sb[:])


def build_program():
    if "nc" in _CACHE:
        return _CACHE["nc"]
    nc = bacc.Bacc("TRN2", target_bir_lowering=False, debug=False, num_devices=8)
    io = {}
    io["xp16"] = nc.dram_tensor("xp16", (C, HP * HP), F16, kind="ExternalInput").ap()
    io["xq"] = nc.dram_tensor("xq", (C, 30 * HP), F16, kind="ExternalInput").ap()
    for nm, shape in [("wv", (C, 9)), ("wq", (C, 9)), ("wk", (C, 9)), ("wo", (C, 9)),
                      ("bv", (C, 1)), ("bq", (C, 1)), ("bk", (C, 1)), ("bo", (C, 1)),
                      ("lng", (C, 1)), ("lnb", (C, 1)), ("w2t", (C, 2)),
                      ("refyx", (2, N))]:
        io[nm] = nc.dram_tensor(nm, shape, F32, kind="ExternalInput").ap()
    io["wot"] = nc.dram_tensor("wot", (C, C), F16, kind="ExternalInput").ap()
    io["vtab"] = nc.dram_tensor("vtab", (PIX, C), F16).ap()
    io["ixy_dram"] = nc.dram_tensor("ixy_dram", (2, N), F32).ap()
    io["y"] = nc.dram_tensor("y", (C, HALF_PIX), F32, kind="ExternalOutput").ap()

    with tile.TileContext(nc) as tc:
        with contextlib.ExitStack() as ctx:
            _emit(nc, tc, ctx, io)
    nc.compile()
    _CACHE["nc"] = nc
    return nc


def host_prep(inputs):
    """Build the 8 per-core input maps from full inputs."""
    x = np.asarray(inputs["x"], np.float32)          # (B, C, H, W)
    xpad = np.pad(x, ((0, 0), (0, 0), (1, 1), (1, 1)))  # (B, C, 58, 58)
    shared = {}
    for nm, src in [("wv", "w_v"), ("wq", "w_q"), ("wk", "w_k"), ("wo", "w_off1")]:
        shared[nm] = np.asarray(inputs[src], np.float32).reshape(C, 9)
    for nm, src in [("bv", "b_v"), ("bq", "b_q"), ("bk", "b_k"), ("bo", "b_off1"),
                    ("lng", "ln_g"), ("lnb", "ln_b")]:
        shared[nm] = np.asarray(inputs[src], np.float32).reshape(C, 1)
    shared["w2t"] = np.ascontiguousarray(np.asarray(inputs["w_off2"], np.float32).T)
    shared["wot"] = np.ascontiguousarray(
        np.asarray(inputs["w_out"], np.float32).T).astype(np.float16)   # (C,C) [c,o]
    ry = (np.arange(KH, dtype=np.float32) + 0.5) / KH * 2 - 1
    rx = (np.arange(KW, dtype=np.float32) + 0.5) / KW * 2 - 1
    refyx = np.stack([np.repeat(ry, KW), np.tile(rx, KH)])   # (2, 784), row0=y
    shared["refyx"] = np.ascontiguousarray(refyx, dtype=np.float32)

    in_maps = []
    for core in range(8):
        b, half = core // 2, core % 2
        m = dict(shared)
        xb = xpad[b]
        m["xp16"] = np.ascontiguousarray(xb.reshape(C, HP * HP)).astype(np.float16)
        r0 = half * HALF_ROWS
        m["xq"] = np.ascontiguousarray(
            xb[:, r0:r0 + 30, :].reshape(C, 30 * HP)).astype(np.float16)
        in_maps.append(m)
    return in_maps


def assemble(results):
    y = np.empty((B, C, H, W), np.float32)
    for core in range(8):
        b, half = core // 2, core % 2
        part = results[core]["y"].reshape(C, HALF_ROWS, W)
        y[b, :, half * HALF_ROWS:(half + 1) * HALF_ROWS, :] = part
    return y


def run(inputs, trace=False):
    nc = build_program()
    in_maps = host_prep(inputs)
    res = run_bass_kernel_spmd(nc, in_maps, core_ids=list(range(8)), trace=trace)
    return assemble(res.results), res


def kernel(**inputs):
    out, _ = run(inputs, trace=False)
    return out
